# revision 1
# baseline (speedup 1.0000x reference)
"""Trainium2 Bass kernel for nn_CombinedN2NWaveletLoss — PE-conv redesign.

Layout: transposed ("T"): partitions carry image COLUMNS, free dim carries
image rows. Per core: 4 images.

- Host: extracts checkerboard phases p0=(even,even), p3=(odd,odd) (the other
  two phases are never used), transposes, converts to fp16, stages the two
  128-column windows (blk0 c0..127, blk1 c128..255) with duplicated edge rows
  for the vertical-upsample clamp.
- DVE: vertical upsample along the free dim: A'[c, 2k] = (p0[k-1]/3 + p0[k]),
  A'[c, 2k+1] = (p0[k+1]/3 + p0[k]) -- stored at 4/3 x true scale; the 0.75
  is folded into every PE stationary.
- PE: horizontal upsample + 3x3 conv FUSED as banded matmuls. N_dy = K_dy @ U
  (K_dy = conv row-kernel matrix, U = bilinear 2x upsample matrix) gives, per
  output column J, a 3-tap stencil over A'-columns. M-chunks = 64 even +
  64 odd output half-columns j' in [64t, 64t+64); the needed c-band
  [64t-1, 64t+66) fits one 128-partition window: t0 -> blk0, t1 -> Wa
  (c62..189), t2 -> Wb (c126..253), t3 -> blk1. Wa/Wb are DMA-duplicated.
  3 accumulating matmuls (dy = -1,0,1 via mov free-offset) per chunk.
- g1/g2 also via PE (2-tap banded U stationaries), accumulated with a -I x out
  matmul so PSUM holds g - out directly; ACT Square+accum / DVE
  tensor_tensor_reduce produce the N2N sums without materializing diffs.
- Eviction: ACT Relu (psum->fp16) + DVE min(.,1) 4x = clip.
- Wavelet: phases are pre-split by the M-chunk layout; DMA shuffles re-align
  partitions per level; details via TT; min(|.|,thr) via fused TS
  (abs_max, min) + TS accumulate.
"""

import numpy as np

B_TOTAL = 32
N_CORES = 8
IMG = 4
H = W = 512
HC = 256
THRESHOLD = 50.0 / 255.0
GAMMA = 2.0
WAVELET_WEIGHT = 0.05
WIN = (0, 62, 126, 128)      # K-window start c per chunk t
NACC = 52                    # 8 d1, 8 d2, 12 lvl1, 12 lvl2, 12 lvl3 (3/img)

# acc slots: 0..7 d1, 8..15 d2, 16+3m+k lvl1, 28+3m+k lvl2, 40+3m+k lvl3
_CACHE = {}


def _upsample_matrix():
    U = np.zeros((H, HC), dtype=np.float64)
    for j in range(H):
        src = (j + 0.5) / 2.0 - 0.5
        k0 = int(np.floor(src))
        frac = src - k0
        for k, wgt in ((k0, 1 - frac), (k0 + 1, frac)):
            U[j, min(max(k, 0), HC - 1)] += wgt
    return U


def _build_stats(w):
    """[128, 17*128] fp16 stationary pack: 12 conv (t,dyi), 4 g (t), 1 -I."""
    U = _upsample_matrix()
    wm = np.asarray(w, dtype=np.float64).reshape(3, 3)
    Nm = []
    for dy in (-1, 0, 1):
        K = np.zeros((H, H))
        for j in range(H):
            for dx in (-1, 0, 1):
                if 0 <= j + dx < H:
                    K[j, j + dx] = wm[dy + 1, dx + 1]
        Nm.append(K @ U)
    out = np.zeros((128, 17 * 128), dtype=np.float32)

    def brev6(q):
        return int(f"{q:06b}"[::-1], 2)

    def lanes(t):
        # bit-reversed lane->column map: keeps every wavelet level's
        # even/odd column split partition-contiguous (lanes q<32 = even u,
        # recursively), so level shuffles are plain contiguous DMAs.
        idx = np.empty(128, dtype=np.int64)
        for m in range(128):
            jp = 64 * t + brev6(m % 64)
            idx[m] = 2 * jp + (0 if m < 64 else 1)
        return idx

    k = 0
    for t in range(4):
        J = lanes(t)
        for i in range(3):
            out[:, k * 128:(k + 1) * 128] = 0.75 * Nm[i][J, WIN[t]:WIN[t] + 128].T
            k += 1
    for t in range(4):
        J = lanes(t)
        out[:, k * 128:(k + 1) * 128] = 0.75 * U[J, WIN[t]:WIN[t] + 128].T
        k += 1
    out[:, k * 128:(k + 1) * 128] = -np.eye(128)
    return out.astype(np.float16)


def _build():
    import concourse.bass as bass
    import concourse.mybir as mybir
    import concourse.tile as tile
    from contextlib import ExitStack

    dt = mybir.dt
    Alu = mybir.AluOpType
    Act = mybir.ActivationFunctionType
    F16, F32 = dt.float16, dt.float32

    T = THRESHOLD
    THR = (T / 4 * 2, T / 2 * 4, T * 8)   # stored-scale thresholds lvl 1..3

    nc = bass.Bass("TRN2", target_bir_lowering=False, debug=False,
                   num_devices=N_CORES)
    xsh = nc.dram_tensor("xs", [128, 2, 2, IMG, 258], F16, kind="ExternalInput")
    sth = nc.dram_tensor("st", [128, 17 * 128], F16, kind="ExternalInput")
    outh = nc.dram_tensor("res", [128, NACC], F32, kind="ExternalOutput")

    with tile.TileContext(nc) as tc, ExitStack() as ctx:
        v = nc.vector
        sc = nc.scalar
        pl = nc.gpsimd

        pp = ctx.enter_context(tc.tile_pool(name="persist", bufs=1))
        xst = pp.tile([128, 2, 2, IMG, 258], F16, tag="xst")
        stats = pp.tile([128, 17 * 128], F16, tag="stats")
        qt = pp.tile([128, 2, 2, IMG, 258], F16, tag="qt")
        # parity planes: [..., 0, r] = A'[2r] (+guard r=256,257),
        #                [..., 1, r] = A'[2r-1] (guards r=0, 257)
        Ag = pp.tile([128, 2, IMG, 2, 258], F16, tag="Ag")
        Bg = pp.tile([128, 2, IMG, 2, 258], F16, tag="Bg")
        AgW = pp.tile([128, 2, IMG, 2, 258], F16, tag="AgW")
        BgW = pp.tile([128, 2, IMG, 2, 258], F16, tag="BgW")
        out = pp.tile([128, IMG, 4, 512], F16, tag="out")
        Ee = pp.tile([128, IMG, 2, 512], F16, tag="Ee")
        Oo = pp.tile([128, IMG, 2, 512], F16, tag="Oo")
        sw = pp.tile([128, IMG, 2, 512], F16, tag="sw")
        dw = pp.tile([128, IMG, 2, 512], F16, tag="dw")
        ll1 = pp.tile([128, IMG, 2, 256], F16, tag="ll1")
        det1 = pp.tile([128, IMG, 3, 2, 256], F16, tag="det1")
        E2 = pp.tile([128, IMG, 256], F16, tag="E2")
        O2 = pp.tile([128, IMG, 256], F16, tag="O2")
        sw2 = pp.tile([128, IMG, 256], F16, tag="sw2")
        dw2 = pp.tile([128, IMG, 256], F16, tag="dw2")
        ll2 = pp.tile([128, IMG, 128], F16, tag="ll2")
        det2 = pp.tile([128, 3, IMG, 128], F16, tag="det2")
        E3 = pp.tile([128, IMG, 128], F16, tag="E3")
        O3 = pp.tile([128, IMG, 128], F16, tag="O3")
        sw3 = pp.tile([128, IMG, 128], F16, tag="sw3")
        dw3 = pp.tile([128, IMG, 128], F16, tag="dw3")
        det3 = pp.tile([128, 3, IMG, 64], F16, tag="det3")
        acc = pp.tile([128, NACC], F32, tag="acc")
        deadA = pp.tile([128, 2, 512], F16, tag="deadA")
        deadV = pp.tile([128, 2, 512], F16, tag="deadV")
        warm = pp.tile([128, 512], F16, tag="warm")

        ppre = ctx.enter_context(tc.tile_pool(name="ppre", bufs=2, space="PSUM"))
        pg = ctx.enter_context(tc.tile_pool(name="pg", bufs=2, space="PSUM"))

        # ---------------- input DMAs (fine-grained, 2 queues) ----------
        nc.scalar.dma_start(out=stats[:, :], in_=sth.ap())
        for si in range(2):
            for w in range(2):
                q = nc.sync if (si + w) % 2 == 0 else nc.scalar
                q.dma_start(out=xst[:, si, w, :, :], in_=xsh.ap()[:, si, w])
        v.memset(acc[:, :], 0.0)
        v.memset(warm[:, :], 0.0)

        # PE warm-up: keep the tensor engine busy while inputs land
        # (borrows a rotating ppre slot; released before the first conv)
        wps = ppre.tile([128, 2, 512], F32, name="wps", tag="pre")
        for _ in range(9):
            nc.tensor.matmul(wps[:, 0, :], warm[:, 0:128], warm[:, :],
                             start=True, stop=True)

        # ---------------- vertical upsample (parity planes, 2x) ----------
        for g_ in (Ag, Bg):
            v.memset(g_[:, :, :, 0, 256:258], 0.0)   # A'[512] guard + pad
            v.memset(g_[:, :, :, 1, 0:1], 0.0)       # A'[-1] guard
            v.memset(g_[:, :, :, 1, 257:258], 0.0)
        for si, g_ in ((0, Ag),):
            for w in range(2):
                v.tensor_scalar(out=qt[:, si, w], in0=xst[:, si, w],
                                scalar1=1.0 / 3.0, scalar2=None, op0=Alu.mult)
                # A'[2k] = q[k] + x[k+1]; A'[2k+1] = q[k+2] + x[k+1]
                v.tensor_tensor(out=g_[:, w, :, 0, 0:256],
                                in0=qt[:, si, w, :, 0:256],
                                in1=xst[:, si, w, :, 1:257], op=Alu.add)
                v.tensor_tensor(out=g_[:, w, :, 1, 1:257],
                                in0=qt[:, si, w, :, 2:258],
                                in1=xst[:, si, w, :, 1:257], op=Alu.add)

        def emit_vert(si, g_):
            for w in range(2):
                v.tensor_scalar(out=qt[:, si, w], in0=xst[:, si, w],
                                scalar1=1.0 / 3.0, scalar2=None, op0=Alu.mult)
                v.tensor_tensor(out=g_[:, w, :, 0, 0:256],
                                in0=qt[:, si, w, :, 0:256],
                                in1=xst[:, si, w, :, 1:257], op=Alu.add)
                v.tensor_tensor(out=g_[:, w, :, 1, 1:257],
                                in0=qt[:, si, w, :, 2:258],
                                in1=xst[:, si, w, :, 1:257], op=Alu.add)

        def emit_windows(src, dst, q1, q2):
            q1.dma_start(out=dst[0:66, 0], in_=src[62:128, 0])
            q2.dma_start(out=dst[66:128, 0], in_=src[0:62, 1])
            q1.dma_start(out=dst[0:2, 1], in_=src[126:128, 0])
            q2.dma_start(out=dst[2:128, 1], in_=src[0:126, 1])

        emit_vert(1, Bg)
        emit_windows(Ag, AgW, nc.sync, nc.sync)
        emit_windows(Bg, BgW, nc.scalar, nc.scalar)

        movs_A = (Ag, AgW, AgW, Ag)
        movs_B = (Bg, BgW, BgW, Bg)
        wsl = (0, 0, 1, 1)   # window slot within the mov tile, by real t
        # physical slot p in the out tile -> real chunk t; pairs (t0,t3) and
        # (t1,t2) so the first psum tile of each image has no window deps
        TP = (0, 3, 1, 2)
        negI = stats[:, 16 * 128:17 * 128]

        # mov slices per (dy, out-parity): out i=2n+par needs A'[2n+par+dy]
        # dy=-1: E<-o[0:256], O<-e[0:256]; dy=0: E<-e[0:256], O<-o[1:257];
        # dy=+1: E<-o[1:257],  O<-e[1:257]
        MOVSL = {(-1, 0): (1, 0, 256), (-1, 1): (0, 0, 256),
                 (0, 0): (0, 0, 256), (0, 1): (1, 1, 257),
                 (1, 0): (1, 1, 257), (1, 1): (0, 1, 257)}

        def conv_img(m, half):
            pre = ppre.tile([128, 2, 512], F32, name="pre", tag="pre")
            for ti in range(2):
                t = TP[2 * half + ti]
                mv = movs_A[t]
                for par in range(2):          # psum cols [0:256]=even i
                    for dyi, dy in enumerate((-1, 0, 1)):
                        pp_, lo, hi = MOVSL[(dy, par)]
                        nc.tensor.matmul(
                            pre[:, ti, 256 * par:256 * par + 256],
                            stats[:, (3 * t + dyi) * 128:(3 * t + dyi + 1) * 128],
                            mv[:, wsl[t], m, pp_, lo:hi],
                            start=(dyi == 0), stop=(dyi == 2))
            return pre

        def g_img(m, half, movs, d_slot, use_act):
            gp = pg.tile([128, 2, 512], F32, name="gp", tag="g")
            for ti in range(2):
                p = 2 * half + ti
                t = TP[p]
                for par in range(2):
                    pp_, lo, hi = MOVSL[(0, par)]
                    cols = slice(256 * par, 256 * par + 256)
                    nc.tensor.matmul(gp[:, ti, cols],
                                     stats[:, (12 + t) * 128:(13 + t) * 128],
                                     movs[t][:, wsl[t], m, pp_, lo:hi],
                                     start=True, stop=False)
                    nc.tensor.matmul(gp[:, ti, cols], negI,
                                     out[:, m, p, cols],
                                     start=False, stop=True)
            if use_act:
                # psum holds g - out; ACT Square + accum
                sc.activation(out=deadA[:, :, :], in_=gp[:, :, :],
                              func=Act.Square,
                              accum_out=acc[:, d_slot:d_slot + 1])
            else:
                # DVE: d = (g - out-psum) - 0... psum alone; square via TTR
                v.tensor_copy(deadV[:, :, :], gp[:, :, :])
                v.tensor_tensor_reduce(
                    out=deadV[:, :, :], in0=deadV[:, :, :],
                    in1=deadV[:, :, :], scale=1.0, scalar=0.0,
                    op0=Alu.mult, op1=Alu.add,
                    accum_out=acc[:, d_slot:d_slot + 1])

        DQ = (nc.sync, nc.scalar, nc.sync)

        gq = []

        def flush_g():
            while gq:
                mm, hh = gq.pop(0)
                g_img(mm, hh, movs_A, mm * 2 + hh, True)
                g_img(mm, hh, movs_B, 8 + mm * 2 + hh, True)

        for m in range(IMG):
            for half in range(2):
                pre = conv_img(m, half)
                v.tensor_scalar(out=out[:, m, 2 * half:2 * half + 2, :],
                                in0=pre[:, :, :], scalar1=0.0, scalar2=1.0,
                                op0=Alu.max, op1=Alu.min)
                gq.append((m, half))
            # wavelet phase shuffles for img m; u = 64t + brev6(q):
            # Ee th slot 0 <- real t0 (p=0), th 1 <- real t2 (p=3);
            # Ee lanes 64: <- odd real t (t1 -> p=2, t3 -> p=1)
            for k, (dst, pl_, ph, psl) in enumerate((
                    (Ee, slice(0, 64), 0, 0), (Ee, slice(0, 64), 1, 3),
                    (Ee, slice(64, 128), 0, 2), (Ee, slice(64, 128), 1, 1),
                    (Oo, slice(0, 64), 0, 0), (Oo, slice(0, 64), 1, 3),
                    (Oo, slice(64, 128), 0, 2), (Oo, slice(64, 128), 1, 1))):
                src = slice(0, 64) if dst is Ee else slice(64, 128)
                DQ[(m + k) % 3].dma_start(out=dst[pl_, m, ph, :],
                                          in_=out[src, m, psl, :])
            # per-image wavelet level 1; i-dim inside out/Ee/Oo is
            # parity-split [e(256)|o(256)] so row pairs are the two halves
            v.tensor_tensor(out=sw[:, m, :, :], in0=Ee[:, m, :, :],
                            in1=Oo[:, m, :, :], op=Alu.add)
            v.tensor_tensor(out=dw[:, m, :, :], in0=Ee[:, m, :, :],
                            in1=Oo[:, m, :, :], op=Alu.subtract)
            swv = sw[:, m, :, :].rearrange("p b (h k) -> p b h k", h=2)
            dwv = dw[:, m, :, :].rearrange("p b (h k) -> p b h k", h=2)
            v.tensor_tensor(out=ll1[:, m, :, :], in0=swv[:, :, 0, :],
                            in1=swv[:, :, 1, :], op=Alu.add)
            v.tensor_tensor(out=det1[:, m, 0, :, :], in0=dwv[:, :, 0, :],
                            in1=dwv[:, :, 1, :], op=Alu.add)
            v.tensor_tensor(out=det1[:, m, 1, :, :], in0=swv[:, :, 0, :],
                            in1=swv[:, :, 1, :], op=Alu.subtract)
            v.tensor_tensor(out=det1[:, m, 2, :, :], in0=dwv[:, :, 0, :],
                            in1=dwv[:, :, 1, :], op=Alu.subtract)
            sc.activation(out=det1[:, m, :, :, :], in_=det1[:, m, :, :, :],
                          func=Act.Abs)
            v.tensor_scalar(out=det1[:, m, :, :, :], in0=det1[:, m, :, :, :],
                            scalar1=THR[0], scalar2=None,
                            op0=Alu.min, op1=Alu.add,
                            accum_out=acc[:, 16 + m:17 + m])
            flush_g()

        # ---- lvl2 (global tail): E2[32tv+s] <- ll1[(tv&1)*64+s, :, tv>>1]
        for tv in range(4):
            sb = (tv & 1) * 64
            th = tv >> 1
            DQ[tv % 3].dma_start(out=E2[32 * tv:32 * tv + 32, :, :],
                                 in_=ll1[sb:sb + 32, :, th, :])
            DQ[(tv + 1) % 3].dma_start(out=O2[32 * tv:32 * tv + 32, :, :],
                                       in_=ll1[sb + 32:sb + 64, :, th, :])
        v.tensor_tensor(out=sw2[:, :, :], in0=E2[:, :, :], in1=O2[:, :, :],
                        op=Alu.add)
        v.tensor_tensor(out=dw2[:, :, :], in0=E2[:, :, :], in1=O2[:, :, :],
                        op=Alu.subtract)
        s2r = sw2[:, :, :].rearrange("p a (k two) -> p a k two", two=2)
        d2r = dw2[:, :, :].rearrange("p a (k two) -> p a k two", two=2)
        v.tensor_tensor(out=ll2[:, :, :], in0=s2r[..., 0], in1=s2r[..., 1],
                        op=Alu.add)
        v.tensor_tensor(out=det2[:, 0, :, :], in0=d2r[..., 0],
                        in1=d2r[..., 1], op=Alu.add)
        v.tensor_tensor(out=det2[:, 1, :, :], in0=s2r[..., 0],
                        in1=s2r[..., 1], op=Alu.subtract)
        v.tensor_tensor(out=det2[:, 2, :, :], in0=d2r[..., 0],
                        in1=d2r[..., 1], op=Alu.subtract)
        sc.activation(out=det2[:, :, :, :], in_=det2[:, :, :, :],
                      func=Act.Abs)
        v.tensor_scalar(out=det2[:, :, :, :], in0=det2[:, :, :, :],
                        scalar1=THR[1], scalar2=None,
                        op0=Alu.min, op1=Alu.add,
                        accum_out=acc[:, 28:29])
        # ---- lvl3
        for tv in range(4):
            DQ[tv % 3].dma_start(out=E3[16 * tv:16 * tv + 16, :, :],
                                 in_=ll2[32 * tv:32 * tv + 16, :, :])
            DQ[(tv + 1) % 3].dma_start(out=O3[16 * tv:16 * tv + 16, :, :],
                                       in_=ll2[32 * tv + 16:32 * tv + 32, :, :])
        v.tensor_tensor(out=sw3[0:64, :, :], in0=E3[0:64, :, :],
                        in1=O3[0:64, :, :], op=Alu.add)
        v.tensor_tensor(out=dw3[0:64, :, :], in0=E3[0:64, :, :],
                        in1=O3[0:64, :, :], op=Alu.subtract)
        s3r = sw3[:, :, :].rearrange("p a (k two) -> p a k two", two=2)
        d3r = dw3[:, :, :].rearrange("p a (k two) -> p a k two", two=2)
        v.tensor_tensor(out=det3[0:64, 0, :, :], in0=d3r[0:64, :, :, 0],
                        in1=d3r[0:64, :, :, 1], op=Alu.add)
        v.tensor_tensor(out=det3[0:64, 1, :, :], in0=s3r[0:64, :, :, 0],
                        in1=s3r[0:64, :, :, 1], op=Alu.subtract)
        v.tensor_tensor(out=det3[0:64, 2, :, :], in0=d3r[0:64, :, :, 0],
                        in1=d3r[0:64, :, :, 1], op=Alu.subtract)
        sc.activation(out=det3[0:64, :, :, :], in_=det3[0:64, :, :, :],
                      func=Act.Abs)
        v.tensor_scalar(out=det3[0:64, :, :, :], in0=det3[0:64, :, :, :],
                        scalar1=THR[2], scalar2=None,
                        op0=Alu.min, op1=Alu.add,
                        accum_out=acc[0:64, 40:41])

        nc.scalar.dma_start(out=outh.ap(), in_=acc[:, :])

    import os
    if not os.environ.get("SKIP_WAIT_SPLIT"):
        _split_multiwaits(nc, mybir)
    return nc


def _split_multiwaits(nc, mybir):
    """HW instructions support exactly ONE sync-wait; split extras into
    standalone Drains (same post-pass as the previous kernel)."""
    for f in nc.m.functions:
        for bb in f.blocks:
            i = 0
            while i < len(bb.instructions):
                ins = bb.instructions[i]
                si = getattr(ins, "sync_info", None)
                if si is not None and si.on_wait and len(si.on_wait) > 1:
                    waits = list(si.on_wait)
                    for w in waits[:-1]:
                        d = mybir.InstDrain(
                            name=nc.get_next_instruction_name(),
                            ins=[], outs=[], bass_is_fusable=False)
                        d.engine = ins.engine
                        d.sync_info = mybir.SyncInfo(on_wait=[w], on_update=[])
                        bb.instructions.insert(i, d)
                        i += 1
                    ins.sync_info = mybir.SyncInfo(
                        on_wait=[waits[-1]], on_update=list(si.on_update))
                i += 1


def _get_nc():
    if "nc" not in _CACHE:
        _CACHE["nc"] = _build()
    return _CACHE["nc"]


def make_in_maps(noisy_input, weight):
    x = np.asarray(noisy_input, dtype=np.float32).reshape(B_TOTAL, H, W)
    stats = _build_stats(weight)
    maps = []
    for c in range(N_CORES):
        xs = np.zeros((128, 2, 2, IMG, 258), dtype=np.float16)
        for m in range(IMG):
            img = x[c * IMG + m]
            for s, ph in enumerate((img[0::2, 0::2], img[1::2, 1::2])):
                pt = np.ascontiguousarray(ph.T).astype(np.float16)  # [c, r]
                st = np.concatenate([pt[:, :1], pt, pt[:, -1:]], axis=1)
                xs[:, s, 0, m, :] = st[0:128]
                xs[:, s, 1, m, :] = st[128:256]
        maps.append({"xs": xs, "st": stats})
    return maps


def _host_combine(parts):
    d1 = d2 = 0.0
    wav = np.zeros(3)
    for p in parts:
        q = p.astype(np.float64)
        d1 += q[:, 0:8].sum()
        d2 += q[:, 8:16].sum()
        wav[0] += q[:, 16:20].sum()
        wav[1] += q[:, 28:29].sum()
        wav[2] += q[0:64, 40:41].sum()
    N = B_TOTAL * H * W
    reg = d1 / N
    rec = d2 / N
    wtot = 0.0
    for j in (1, 2, 3):
        lvl = 3 - j + 1
        Nj = B_TOTAL * (H // 2 ** j) ** 2 * 3
        wtot += (1.0 / lvl) * (wav[j - 1] / (2.0 ** j)) / Nj
    return np.float32(rec + GAMMA * reg + WAVELET_WEIGHT * wtot)


def kernel(noisy_input, weight):
    from concourse.bass_utils import run_bass_kernel_spmd
    nc = _get_nc()
    in_maps = make_in_maps(noisy_input, weight)
    res = run_bass_kernel_spmd(nc, in_maps, list(range(N_CORES)))
    return _host_combine([r["res"] for r in res.results])



# revision 22
# speedup vs baseline: 1.0546x; 1.0546x over previous
"""Trainium2 Bass kernel for nn_CombinedN2NWaveletLoss — v2 redesign.

Loss algebra: rec + 2*reg = 3*mean((out-c)^2) + (2/3)*mean((U delta U^T)^2)
with c = U qc U^T / 3, qc = 2*p0 + p3, delta = p0 - p3.  The delta term is
computed on the coarse grid via the Gram quadratic form sum(delta' * (G d G))
with G = U^T U (tridiagonal: diag 1.25, off 0.375, clamped edges); the row
factor (Gv/1.25, exact at the clamped edges thanks to dup guards) is staged
from the host, the column factor is 2 main + 2 halo PE matmuls per image.

Layout: transposed; partitions carry image COLUMNS, free dim carries rows.
The vertical (row) 2x upsample is folded into the PE stationaries as 3
coarse row-taps (T[par][dn] = sum_dy rho[par,dy,dn] * (K_dy @ U)), so the
conv consumes raw p0 directly.  Spurious conv zero-pad terms at fine rows
0/511 are removed by an extra 1-column matmul inside each accumulation
group.  Eviction: ACT Relu (psum->fp16) + one DVE min(.,1) per image at 4x.
The c field uses 2 row-taps (1/3 folded in) plus -I matmuls on clipped out,
evicted with ACT Square+accum.  Wavelet lvl1: batched shuffle DMAs + DVE
butterflies + ACT Abs + DVE min/accum.  lvl2/lvl3: entirely on PE via
(P_E +- P_O) permutation-butterfly stationaries with row-pair accumulation,
ACT Abs-fused psum evictions, DVE min/accum.
"""

import numpy as np

B_TOTAL = 32
N_CORES = 8
IMG = 4
H = W = 512
HC = 256
THRESHOLD = 50.0 / 255.0
GAMMA = 2.0
WAVELET_WEIGHT = 0.05
WIN = (0, 62, 126, 128)          # stationary col-window start per chunk t
PAIRS = ((0, 3), (1, 2))         # chunk pairs per unit half
MOVP = (0, 2, 3, 1)              # p0 moving plane per chunk t
CPLANE = (4, 6, 7, 5)            # qc moving plane per chunk t
NPLANE = 12                      # p0 b0,b1,Wa,Wb; qc b0,b1,Wa,Wb; d b0,b1; d' b0,b1
NACC = 16                        # 0..7 c-units, 8..9 lvl1 pairs, 10 lvl2, 11 lvl3, 12..15 delta
CTAPS = (((-1, 0.25), (0, 0.75)), ((0, 0.75), (1, 0.25)))

# stationary pack indices
def st_conv(t, par, dni):
    return (t * 6 + par * 3 + dni) * 128

def st_c(t, par, j):
    return (24 + t * 4 + par * 2 + j) * 128

ST_I = 40 * 128

def st_g(ch, h):
    return (41 + ch * 2 + h) * 128

def st_corr(t, e):
    return (45 + t * 2 + e) * 128

def st_l2(sd, th, neg):              # sd: 0=sum(P_E+P_O) 1=dif; neg: rp sign
    return (53 + sd * 2 + th + 4 * neg) * 128

def st_l3(sd, neg):
    return (61 + sd + 2 * neg) * 128

NSTAT = 65

_CACHE = {}


def _brev6(q):
    return int(f"{q:06b}"[::-1], 2)


def _lanes(t):
    idx = np.empty(128, dtype=np.int64)
    for q in range(128):
        jp = 64 * t + _brev6(q % 64)
        idx[q] = 2 * jp + (0 if q < 64 else 1)
    return idx


def _upsample_matrix():
    U = np.zeros((H, HC), dtype=np.float64)
    for j in range(H):
        src = (j + 0.5) / 2.0 - 0.5
        k0 = int(np.floor(src))
        frac = src - k0
        for k, wgt in ((k0, 1 - frac), (k0 + 1, frac)):
            U[j, min(max(k, 0), HC - 1)] += wgt
    return U


def _conv_colops(w):
    U = _upsample_matrix()
    wm = np.asarray(w, dtype=np.float64).reshape(3, 3)
    Nm = []
    for dy in (-1, 0, 1):
        K = np.zeros((H, H))
        for j in range(H):
            for dx in (-1, 0, 1):
                if 0 <= j + dx < H:
                    K[j, j + dx] = wm[dy + 1, dx + 1]
        Nm.append(K @ U)
    return Nm


def _perm_l2(eo):
    """P with P[src_lane, dst] = 1: E2/O2[32tv+s] <- ll1[(tv&1)*64+32*eo+s]."""
    P = np.zeros((128, 128))
    for tv in range(4):
        for s in range(32):
            P[(tv & 1) * 64 + 32 * eo + s, 32 * tv + s] = 1.0
    return P


def _perm_l2_th(eo, th):
    """th-masked: only tv with tv>>1 == th contribute."""
    P = np.zeros((128, 128))
    for tv in range(4):
        if (tv >> 1) != th:
            continue
        for s in range(32):
            P[(tv & 1) * 64 + 32 * eo + s, 32 * tv + s] = 1.0
    return P


def _perm_l3(eo):
    P = np.zeros((128, 128))
    for tv in range(4):
        for s in range(16):
            P[32 * tv + 16 * eo + s, 16 * tv + s] = 1.0
    return P


def _build_stats(w):
    Nm = _conv_colops(w)
    rho = np.zeros((2, 3, 3))
    rho[0, 0, 0] = 0.75; rho[0, 0, 1] = 0.25
    rho[0, 1, 0] = 0.25; rho[0, 1, 1] = 0.75
    rho[0, 2, 1] = 0.75; rho[0, 2, 2] = 0.25
    rho[1, 0, 0] = 0.25; rho[1, 0, 1] = 0.75
    rho[1, 1, 1] = 0.75; rho[1, 1, 2] = 0.25
    rho[1, 2, 1] = 0.25; rho[1, 2, 2] = 0.75
    T = [[sum(rho[par, dyi, dni] * Nm[dyi] for dyi in range(3))
          for dni in range(3)] for par in range(2)]
    U = _upsample_matrix()
    G125 = 1.25 * (U.T @ U)

    out = np.zeros((128, NSTAT * 128), dtype=np.float32)
    for t in range(4):
        J = _lanes(t)
        for par in range(2):
            for dni in range(3):
                k = st_conv(t, par, dni)
                out[:, k:k + 128] = T[par][dni][J, WIN[t]:WIN[t] + 128].T
            for j, (dn, r) in enumerate(CTAPS[par]):
                k = st_c(t, par, j)
                out[:, k:k + 128] = (r / 3.0) * U[J, WIN[t]:WIN[t] + 128].T
        # conv edge corrections (zero-pad spurious terms), 1-col matmuls
        for e, Ncor in ((0, Nm[0]), (1, Nm[2])):
            k = st_corr(t, e)
            out[:, k:k + 128] = -Ncor[J, WIN[t]:WIN[t] + 128].T
    out[:, ST_I:ST_I + 128] = -np.eye(128)
    for ch in range(2):
        a = 128 * ch
        b = 128 * (1 - ch)
        out[:, st_g(ch, 0):st_g(ch, 0) + 128] = G125[a:a + 128, a:a + 128]
        out[:, st_g(ch, 1):st_g(ch, 1) + 128] = G125[b:b + 128, a:a + 128]
    # lvl2/lvl3 butterfly stationaries
    for th in range(2):
        S = _perm_l2_th(0, th)
        O = _perm_l2_th(1, th)
        for neg in range(2):
            sgn = -1.0 if neg else 1.0
            out[:, st_l2(0, th, neg):st_l2(0, th, neg) + 128] = sgn * (S + O)
            out[:, st_l2(1, th, neg):st_l2(1, th, neg) + 128] = sgn * (S - O)
    S3 = _perm_l3(0)
    O3 = _perm_l3(1)
    for neg in range(2):
        sgn = -1.0 if neg else 1.0
        out[:, st_l3(0, neg):st_l3(0, neg) + 128] = sgn * (S3 + O3)
        out[:, st_l3(1, neg):st_l3(1, neg) + 128] = sgn * (S3 - O3)
    return out.astype(np.float16)


def _build():
    import concourse.bass as bass
    import concourse.mybir as mybir
    import concourse.tile as tile
    from contextlib import ExitStack

    dt = mybir.dt
    Alu = mybir.AluOpType
    Act = mybir.ActivationFunctionType
    F16, F32 = dt.float16, dt.float32

    T = THRESHOLD
    THR = (T / 4 * 2, T / 2 * 4, T * 8)

    nc = bass.Bass("TRN2", target_bir_lowering=False, debug=False,
                   num_devices=N_CORES)
    xsh = nc.dram_tensor("xs", [128, NPLANE, IMG, 258], F16, kind="ExternalInput")
    sth = nc.dram_tensor("st", [128, NSTAT * 128], F16, kind="ExternalInput")
    outh = nc.dram_tensor("res", [128, NACC], F32, kind="ExternalOutput")

    with tile.TileContext(nc) as tc, ExitStack() as ctx:
        v = nc.vector
        sc = nc.scalar

        pp = ctx.enter_context(tc.tile_pool(name="persist", bufs=1))
        xst = pp.tile([128, NPLANE, IMG, 258], F16, tag="xst")
        stats = pp.tile([128, NSTAT * 128], F16, tag="stats")
        out = pp.tile([128, 4, IMG, 512], F16, tag="out")     # [slot=t, m]
        Ee = pp.tile([128, 2, IMG, 512], F16, tag="Ee")       # [ph, m]
        Oo = pp.tile([128, 2, IMG, 512], F16, tag="Oo")
        sw = pp.tile([128, 2, IMG, 512], F16, tag="sw")       # [ph, m]
        dw = pp.tile([128, 2, IMG, 512], F16, tag="dw")
        ll1 = pp.tile([128, 2, IMG, 256], F16, tag="ll1")     # [th, m]
        det1 = pp.tile([128, 2, 3, 2, 2, 256], F16, tag="det1")  # [P, b, ph, mp]
        ll2 = pp.tile([128, IMG, 128], F16, tag="ll2")
        det2 = pp.tile([128, 3, IMG, 128], F16, tag="det2")
        det3 = pp.tile([128, 3, IMG, 64], F16, tag="det3")
        deadc = pp.tile([128, 2, 512], F16, tag="deadc")
        deadw = pp.tile([128, 2, 256], F16, tag="deadw")
        deadt = pp.tile([128, 3072], F16, tag="deadt")
        acc = pp.tile([128, NACC], F32, tag="acc")
        warm = pp.tile([128, 512], F16, tag="warm")

        ppre = ctx.enter_context(tc.tile_pool(name="ppre", bufs=2, space="PSUM"))
        pc = ctx.enter_context(tc.tile_pool(name="pc", bufs=2, space="PSUM"))

        # ---------------- input DMAs ----------------
        nc.sync.dma_start(out=stats[:, 0:24 * 128], in_=sth.ap()[:, 0:24 * 128])
        nc.sync.dma_start(out=xst[:, 0:4, :, :], in_=xsh.ap()[:, 0:4])
        nc.scalar.dma_start(out=xst[:, 4:8, :, :], in_=xsh.ap()[:, 4:8])
        nc.scalar.dma_start(out=xst[:, 8:12, :, :], in_=xsh.ap()[:, 8:12])
        nc.scalar.dma_start(out=stats[:, 24 * 128:], in_=sth.ap()[:, 24 * 128:])

        v.memset(warm[:, :], 0.0)
        v.memset(acc[:, :], 0.0)

        # PE warm-up while inputs land
        wps = ppre.tile([128, 2, 512], F32, name="wps", tag="pre")
        for _ in range(7):
            nc.tensor.matmul(wps[:, 0, :], warm[:, 0:128], warm[:, :],
                             start=True, stop=True)

        # ---------------- conv units ----------------
        def conv_unit(m, half):
            pre = ppre.tile([128, 2, 512], F32, name="pre", tag="pre")
            for ti, t in enumerate(PAIRS[half]):
                mv = xst[:, MOVP[t], m, :]
                for par in range(2):
                    # edge-correction 1-col matmul folded into the group:
                    # par0 fixes fine row 0 (free slot 0), par1 row 511 (511)
                    nc.tensor.matmul(
                        pre[:, ti, 256 * par:256 * par + 256],
                        stats[:, st_conv(t, par, 0):st_conv(t, par, 0) + 128],
                        mv[:, 0:256], start=True, stop=False)
                    nc.tensor.matmul(
                        pre[:, ti, 256 * par:256 * par + 256],
                        stats[:, st_conv(t, par, 1):st_conv(t, par, 1) + 128],
                        mv[:, 1:257], start=False, stop=False)
                    if par == 0:
                        nc.tensor.matmul(
                            pre[:, ti, 0:1],
                            stats[:, st_corr(t, 0):st_corr(t, 0) + 128],
                            mv[:, 1:2], start=False, stop=False)
                    else:
                        nc.tensor.matmul(
                            pre[:, ti, 511:512],
                            stats[:, st_corr(t, 1):st_corr(t, 1) + 128],
                            mv[:, 256:257], start=False, stop=False)
                    nc.tensor.matmul(
                        pre[:, ti, 256 * par:256 * par + 256],
                        stats[:, st_conv(t, par, 2):st_conv(t, par, 2) + 128],
                        mv[:, 2:258], start=False, stop=True)
            for ti, t in enumerate(PAIRS[half]):
                sc.activation(out=out[:, t, m, :], in_=pre[:, ti, :],
                              func=Act.Relu)

        def min_img(m):
            v.tensor_scalar(out=out[:, :, m, :], in0=out[:, :, m, :],
                            scalar1=1.0, scalar2=None, op0=Alu.min)

        # ---------------- c units ----------------
        def c_unit(m, half):
            ct = pc.tile([128, 2, 512], F32, name="ct", tag="ct")
            for ti, t in enumerate(PAIRS[half]):
                mv = xst[:, CPLANE[t], m, :]
                for par in range(2):
                    cols = slice(256 * par, 256 * par + 256)
                    for j, (dn, r) in enumerate(CTAPS[par]):
                        nc.tensor.matmul(ct[:, ti, cols],
                                         stats[:, st_c(t, par, j):st_c(t, par, j) + 128],
                                         mv[:, dn + 1:dn + 257],
                                         start=(j == 0), stop=False)
                    nc.tensor.matmul(ct[:, ti, cols],
                                     stats[:, ST_I:ST_I + 128],
                                     out[:, t, m, cols],
                                     start=False, stop=True)
            sc.activation(out=deadc[:, :, :], in_=ct[:, :, :], func=Act.Square,
                          accum_out=acc[:, 2 * m + half:2 * m + half + 1])

        # ---------------- delta term ----------------
        def delta_img(m):
            Wp = pc.tile([128, 2, 512], F32, name="wp", tag="ct")
            for ch in range(2):
                nc.tensor.matmul(Wp[:, ch, 0:256],
                                 stats[:, st_g(ch, 0):st_g(ch, 0) + 128],
                                 xst[:, 10 + ch, m, 1:257], start=True, stop=False)
                nc.tensor.matmul(Wp[:, ch, 0:256],
                                 stats[:, st_g(ch, 1):st_g(ch, 1) + 128],
                                 xst[:, 11 - ch, m, 1:257], start=False, stop=True)
            v.tensor_tensor(out=deadw[:, :, :], in0=xst[:, 8:10, m, 1:257],
                            in1=Wp[:, :, 0:256], op=Alu.mult)
            v.tensor_scalar(out=deadw[:, :, :], in0=deadw[:, :, :], scalar1=0.0,
                            scalar2=None, op0=Alu.bypass, op1=Alu.add,
                            accum_out=acc[:, 12 + m:13 + m])

        # ---------------- wavelet lvl1 ----------------
        outr = out[:, :, :, :].rearrange("p (sa sb) m f -> p sb sa (m f)",
                                         sa=2, sb=2)

        def shuffle_pair(P):
            fs = slice(1024 * P, 1024 * P + 1024)
            ms = slice(2 * P, 2 * P + 2)
            nc.sync.dma_start(out=Ee[0:64, :, ms, :], in_=outr[0:64, 0, :, fs])
            nc.sync.dma_start(out=Ee[64:128, :, ms, :], in_=outr[0:64, 1, :, fs])
            nc.sync.dma_start(out=Oo[0:64, :, ms, :], in_=outr[64:128, 0, :, fs])
            nc.sync.dma_start(out=Oo[64:128, :, ms, :], in_=outr[64:128, 1, :, fs])

        def lvl1_tt(P):
            ms = slice(2 * P, 2 * P + 2)
            eer = Ee[:, :, ms, :]
            oor = Oo[:, :, ms, :]
            v.tensor_tensor(out=sw[:, :, ms, :], in0=eer, in1=oor, op=Alu.add)
            v.tensor_tensor(out=dw[:, :, ms, :], in0=eer, in1=oor, op=Alu.subtract)
            v.tensor_tensor(out=ll1[:, :, ms, :], in0=sw[:, :, ms, 0:256],
                            in1=sw[:, :, ms, 256:512], op=Alu.add)
            v.tensor_tensor(out=det1[:, P, 0, :, :, :], in0=dw[:, :, ms, 0:256],
                            in1=dw[:, :, ms, 256:512], op=Alu.add)
            v.tensor_tensor(out=det1[:, P, 1, :, :, :], in0=sw[:, :, ms, 0:256],
                            in1=sw[:, :, ms, 256:512], op=Alu.subtract)
            v.tensor_tensor(out=det1[:, P, 2, :, :, :], in0=dw[:, :, ms, 0:256],
                            in1=dw[:, :, ms, 256:512], op=Alu.subtract)

        def lvl1_ts(P):
            flat = det1[:, P, :, :, :, :].rearrange("p b ph m f -> p (b ph m f)")
            sc.activation(out=flat, in_=flat, func=Act.Abs)
            v.tensor_scalar(out=deadt[:, 0:3072], in0=flat, scalar1=THR[0],
                            scalar2=None, op0=Alu.min, op1=Alu.add,
                            accum_out=acc[:, 8 + P:9 + P])

        # ---------------- wavelet lvl2/lvl3 on PE ----------------
        def l1v(th, rp):
            return ll1[:, th, :, :].rearrange(
                "p m (k two) -> p m k two", two=2)[..., rp]

        def lvl2():
            pa = ppre.tile([128, 2, 512], F32, name="pa", tag="pre")
            pb = pc.tile([128, 2, 512], F32, name="pb", tag="ct")
            bands = ((pa, 0, 0, (0, 0)), (pa, 1, 1, (0, 0)),
                     (pb, 0, 0, (0, 1)), (pb, 1, 1, (0, 1)))
            # (tile, slot, sd, rp-neg pattern): band0=ll2(sum,+,+),
            # band1=det2[0](dif,+,+), band2=det2[1](sum,+,-), band3=det2[2](dif,+,-)
            for bi, (tile_, slot, sd, negs) in enumerate(bands):
                dstv = tile_[:, slot, :].rearrange("p (m k) -> p m k", m=IMG)
                k = 0
                for rp in range(2):
                    for th in range(2):
                        nc.tensor.matmul(
                            dstv, stats[:, st_l2(sd, th, negs[rp]):
                                        st_l2(sd, th, negs[rp]) + 128],
                            l1v(th, rp), start=(k == 0), stop=(k == 3))
                        k += 1
            sc.activation(out=ll2[:, :, :],
                          in_=pa[:, 0, :].rearrange("p (m k) -> p m k", m=IMG),
                          func=Act.Copy)
            sc.activation(out=det2[:, 0, :, :],
                          in_=pa[:, 1, :].rearrange("p (m k) -> p m k", m=IMG),
                          func=Act.Abs)
            sc.activation(out=det2[:, 1, :, :],
                          in_=pb[:, 0, :].rearrange("p (m k) -> p m k", m=IMG),
                          func=Act.Abs)
            sc.activation(out=det2[:, 2, :, :],
                          in_=pb[:, 1, :].rearrange("p (m k) -> p m k", m=IMG),
                          func=Act.Abs)
            flat = det2[:, :, :, :].rearrange("p b m f -> p (b m f)")
            v.tensor_scalar(out=deadt[:, 0:1536], in0=flat, scalar1=THR[1],
                            scalar2=None, op0=Alu.min, op1=Alu.add,
                            accum_out=acc[:, 10:11])

        def l2v(rp):
            return ll2[:, :, :].rearrange(
                "p m (k two) -> p m k two", two=2)[..., rp]

        def lvl3():
            p3 = ppre.tile([128, 2, 512], F32, name="p3", tag="pre")
            regions = (p3[:, 0, 0:256], p3[:, 0, 256:512], p3[:, 1, 0:256])
            for b in range(3):
                sd = (1, 0, 1)[b]
                negs = ((0, 0), (0, 1), (0, 1))[b]
                dstv = regions[b].rearrange("p (m k) -> p m k", m=IMG)
                for rp in range(2):
                    nc.tensor.matmul(
                        dstv, stats[:, st_l3(sd, negs[rp]):
                                    st_l3(sd, negs[rp]) + 128],
                        l2v(rp), start=(rp == 0), stop=(rp == 1))
                sc.activation(out=det3[:, b, :, :], in_=dstv, func=Act.Abs)
            flat = det3[:, :, :, :].rearrange("p b m f -> p (b m f)")
            v.tensor_scalar(out=deadt[:, 0:768], in0=flat, scalar1=THR[2],
                            scalar2=None, op0=Alu.min, op1=Alu.add,
                            accum_out=acc[:, 11:12])

        # ---------------- main schedule ----------------
        for m in range(IMG):
            conv_unit(m, 0)
            conv_unit(m, 1)
            min_img(m)
            if m == 1:
                shuffle_pair(0)
                lvl1_tt(0)
                lvl1_ts(0)
            if m == 3:
                shuffle_pair(1)
                lvl1_tt(1)
                lvl1_ts(1)
        for m in range(IMG):
            c_unit(m, 0)
            c_unit(m, 1)
            delta_img(m)
        lvl2()
        lvl3()

        nc.sync.dma_start(out=outh.ap(), in_=acc[:, :])

    import os
    if not os.environ.get("SKIP_WAIT_SPLIT"):
        _split_multiwaits(nc, mybir)
    return nc


def _split_multiwaits(nc, mybir):
    """HW instructions support exactly ONE sync-wait; split extras into
    standalone Drains."""
    for f in nc.m.functions:
        for bb in f.blocks:
            i = 0
            while i < len(bb.instructions):
                ins = bb.instructions[i]
                si = getattr(ins, "sync_info", None)
                if si is not None and si.on_wait and len(si.on_wait) > 1:
                    waits = list(si.on_wait)
                    for w in waits[:-1]:
                        d = mybir.InstDrain(
                            name=nc.get_next_instruction_name(),
                            ins=[], outs=[], bass_is_fusable=False)
                        d.engine = ins.engine
                        d.sync_info = mybir.SyncInfo(on_wait=[w], on_update=[])
                        bb.instructions.insert(i, d)
                        i += 1
                    ins.sync_info = mybir.SyncInfo(
                        on_wait=[waits[-1]], on_update=list(si.on_update))
                i += 1


def _get_nc():
    if "nc" not in _CACHE:
        _CACHE["nc"] = _build()
    return _CACHE["nc"]


def make_in_maps(noisy_input, weight):
    x = np.asarray(noisy_input, dtype=np.float32).reshape(B_TOTAL, H, W)
    stats = _build_stats(weight)
    maps = []
    for c in range(N_CORES):
        xs = np.zeros((128, NPLANE, IMG, 258), dtype=np.float16)
        for m in range(IMG):
            img = x[c * IMG + m]
            p0 = img[0::2, 0::2].astype(np.float64)
            p3 = img[1::2, 1::2].astype(np.float64)
            qc = 2.0 * p0 + p3
            dl = p0 - p3
            g = np.concatenate([dl[:1], dl, dl[-1:]], axis=0)
            dp = g[1:257] + 0.3 * (g[0:256] + g[2:258])   # Gv/1.25 row factor
            for base, ph, wins in ((0, p0, True), (4, qc, True),
                                   (8, dl, False)):
                pt = np.ascontiguousarray(ph.T)          # [col, row]
                st = np.concatenate([pt[:, :1], pt, pt[:, -1:]], axis=1)
                xs[:, base + 0, m, :] = st[0:128]
                xs[:, base + 1, m, :] = st[128:256]
                if wins:
                    xs[:, base + 2, m, :] = st[62:190]   # Wa
                    xs[:, base + 3, m, :] = st[126:254]  # Wb
            dpt = np.ascontiguousarray(dp.T)
            xs[:, 10, m, 1:257] = dpt[0:128]
            xs[:, 11, m, 1:257] = dpt[128:256]
        maps.append({"xs": xs, "st": stats})
    return maps


def _host_combine(parts):
    S_c = S_d = 0.0
    wav = np.zeros(3)
    for p in parts:
        q = p.astype(np.float64)
        S_c += q[:, 0:8].sum()
        wav[0] += q[:, 8:10].sum()
        wav[1] += q[:, 10].sum()
        wav[2] += q[:, 11].sum()
        S_d += q[:, 12:16].sum()
    N = B_TOTAL * H * W
    n2n = (3.0 * S_c + (2.0 / 3.0) * S_d) / N
    wtot = 0.0
    for j in (1, 2, 3):
        lvl = 3 - j + 1
        Nj = B_TOTAL * (H // 2 ** j) ** 2 * 3
        wtot += (1.0 / lvl) * (wav[j - 1] / (2.0 ** j)) / Nj
    return np.float32(n2n + WAVELET_WEIGHT * wtot)


def kernel(noisy_input, weight):
    from concourse.bass_utils import run_bass_kernel_spmd
    nc = _get_nc()
    in_maps = make_in_maps(noisy_input, weight)
    res = run_bass_kernel_spmd(nc, in_maps, list(range(N_CORES)))
    return _host_combine([r["res"] for r in res.results])


# revision 27
# speedup vs baseline: 1.1856x; 1.1243x over previous
"""Trainium2 Bass kernel for nn_CombinedN2NWaveletLoss — v2 redesign.

Loss algebra: rec + 2*reg = 3*mean((out-c)^2) + (2/3)*mean((U delta U^T)^2)
with c = U qc U^T / 3, qc = 2*p0 + p3, delta = p0 - p3.  The delta term is
computed on the coarse grid via the Gram quadratic form sum(delta' * (G d G))
with G = U^T U (tridiagonal: diag 1.25, off 0.375, clamped edges); the row
factor (Gv/1.25, exact at the clamped edges thanks to dup guards) is staged
from the host, the column factor is 2 main + 2 halo PE matmuls per image.

Layout: transposed; partitions carry image COLUMNS, free dim carries rows.
The vertical (row) 2x upsample is folded into the PE stationaries as 3
coarse row-taps (T[par][dn] = sum_dy rho[par,dy,dn] * (K_dy @ U)), so the
conv consumes raw p0 directly.  Spurious conv zero-pad terms at fine rows
0/511 are removed by an extra 1-column matmul inside each accumulation
group.  Eviction: ACT Relu (psum->fp16) + one DVE min(.,1) per image at 4x.
The c field uses 2 row-taps (1/3 folded in) plus -I matmuls on clipped out,
evicted with ACT Square+accum.  Wavelet lvl1: batched shuffle DMAs + DVE
butterflies + ACT Abs + DVE min/accum.  lvl2/lvl3: entirely on PE via
(P_E +- P_O) permutation-butterfly stationaries with row-pair accumulation,
ACT Abs-fused psum evictions, DVE min/accum.
"""

import numpy as np

B_TOTAL = 32
N_CORES = 8
IMG = 4
H = W = 512
HC = 256
THRESHOLD = 50.0 / 255.0
GAMMA = 2.0
WAVELET_WEIGHT = 0.05
WIN = (0, 62, 126, 128)          # stationary col-window start per chunk t
PAIRS = ((0, 3), (1, 2))         # chunk pairs per unit half
MOVP = (0, 2, 3, 1)              # p0 moving plane per chunk t
CPLANE = (4, 6, 7, 5)            # qc moving plane per chunk t
NPLANE = 10                      # p0 b0,b1,Wa,Wb; qc b0,b1,Wa,Wb; d b0,b1
NACC = 16                        # 0..7 c-units, 8..9 lvl1 pairs, 10 lvl2, 11 lvl3, 12..15 delta
CTAPS = (((-1, 0.25), (0, 0.75)), ((0, 0.75), (1, 0.25)))

# stationary pack indices
def st_conv(t, par, dni):
    return (t * 6 + par * 3 + dni) * 128

def st_corr(t, e):
    return (24 + t * 2 + e) * 128

def st_c(t, par, j):
    # Tc[0][0]=Tc[1][1]=(0.25/3)U-window, Tc[0][1]=Tc[1][0]=(0.75/3)U-window:
    # only 2 distinct blocks per chunk.
    big = 1 if (par == 0) == (j == 1) else 0
    return (32 + t * 2 + big) * 128

ST_I = 40 * 128

def st_g(ch, h):
    return (41 + ch * 2 + h) * 128

def st_l2(sd, th, neg):              # sd: 0=sum(P_E+P_O) 1=dif; neg: rp sign
    return (45 + sd * 2 + th + 4 * neg) * 128

def st_l3(sd, neg):
    return (53 + sd + 2 * neg) * 128

NSTAT = 57

_CACHE = {}


def _brev6(q):
    return int(f"{q:06b}"[::-1], 2)


def _lanes(t):
    idx = np.empty(128, dtype=np.int64)
    for q in range(128):
        jp = 64 * t + _brev6(q % 64)
        idx[q] = 2 * jp + (0 if q < 64 else 1)
    return idx


def _upsample_matrix():
    U = np.zeros((H, HC), dtype=np.float64)
    for j in range(H):
        src = (j + 0.5) / 2.0 - 0.5
        k0 = int(np.floor(src))
        frac = src - k0
        for k, wgt in ((k0, 1 - frac), (k0 + 1, frac)):
            U[j, min(max(k, 0), HC - 1)] += wgt
    return U


def _conv_colops(w):
    U = _upsample_matrix()
    wm = np.asarray(w, dtype=np.float64).reshape(3, 3)
    Nm = []
    for dy in (-1, 0, 1):
        K = np.zeros((H, H))
        for j in range(H):
            for dx in (-1, 0, 1):
                if 0 <= j + dx < H:
                    K[j, j + dx] = wm[dy + 1, dx + 1]
        Nm.append(K @ U)
    return Nm


def _perm_l2(eo):
    """P with P[src_lane, dst] = 1: E2/O2[32tv+s] <- ll1[(tv&1)*64+32*eo+s]."""
    P = np.zeros((128, 128))
    for tv in range(4):
        for s in range(32):
            P[(tv & 1) * 64 + 32 * eo + s, 32 * tv + s] = 1.0
    return P


def _perm_l2_th(eo, th):
    """th-masked: only tv with tv>>1 == th contribute."""
    P = np.zeros((128, 128))
    for tv in range(4):
        if (tv >> 1) != th:
            continue
        for s in range(32):
            P[(tv & 1) * 64 + 32 * eo + s, 32 * tv + s] = 1.0
    return P


def _perm_l3(eo):
    P = np.zeros((128, 128))
    for tv in range(4):
        for s in range(16):
            P[32 * tv + 16 * eo + s, 16 * tv + s] = 1.0
    return P


def _build_stats(w):
    Nm = _conv_colops(w)
    rho = np.zeros((2, 3, 3))
    rho[0, 0, 0] = 0.75; rho[0, 0, 1] = 0.25
    rho[0, 1, 0] = 0.25; rho[0, 1, 1] = 0.75
    rho[0, 2, 1] = 0.75; rho[0, 2, 2] = 0.25
    rho[1, 0, 0] = 0.25; rho[1, 0, 1] = 0.75
    rho[1, 1, 1] = 0.75; rho[1, 1, 2] = 0.25
    rho[1, 2, 1] = 0.25; rho[1, 2, 2] = 0.75
    T = [[sum(rho[par, dyi, dni] * Nm[dyi] for dyi in range(3))
          for dni in range(3)] for par in range(2)]
    U = _upsample_matrix()
    G125 = 1.25 * (U.T @ U)

    out = np.zeros((128, NSTAT * 128), dtype=np.float32)
    for t in range(4):
        J = _lanes(t)
        for par in range(2):
            for dni in range(3):
                k = st_conv(t, par, dni)
                out[:, k:k + 128] = T[par][dni][J, WIN[t]:WIN[t] + 128].T
        for big, r in ((0, 0.25), (1, 0.75)):
            k = (32 + t * 2 + big) * 128
            out[:, k:k + 128] = (r / 3.0) * U[J, WIN[t]:WIN[t] + 128].T
        # conv edge corrections (zero-pad spurious terms), 1-col matmuls
        for e, Ncor in ((0, Nm[0]), (1, Nm[2])):
            k = st_corr(t, e)
            out[:, k:k + 128] = -Ncor[J, WIN[t]:WIN[t] + 128].T
    out[:, ST_I:ST_I + 128] = -np.eye(128)
    for ch in range(2):
        a = 128 * ch
        b = 128 * (1 - ch)
        out[:, st_g(ch, 0):st_g(ch, 0) + 128] = G125[a:a + 128, a:a + 128]
        out[:, st_g(ch, 1):st_g(ch, 1) + 128] = G125[b:b + 128, a:a + 128]
    # lvl2/lvl3 butterfly stationaries
    for th in range(2):
        S = _perm_l2_th(0, th)
        O = _perm_l2_th(1, th)
        for neg in range(2):
            sgn = -1.0 if neg else 1.0
            out[:, st_l2(0, th, neg):st_l2(0, th, neg) + 128] = sgn * (S + O)
            out[:, st_l2(1, th, neg):st_l2(1, th, neg) + 128] = sgn * (S - O)
    S3 = _perm_l3(0)
    O3 = _perm_l3(1)
    for neg in range(2):
        sgn = -1.0 if neg else 1.0
        out[:, st_l3(0, neg):st_l3(0, neg) + 128] = sgn * (S3 + O3)
        out[:, st_l3(1, neg):st_l3(1, neg) + 128] = sgn * (S3 - O3)
    return out.astype(np.float16)


def _build():
    import concourse.bass as bass
    import concourse.mybir as mybir
    import concourse.tile as tile
    from contextlib import ExitStack

    dt = mybir.dt
    Alu = mybir.AluOpType
    Act = mybir.ActivationFunctionType
    F16, F32 = dt.float16, dt.float32

    T = THRESHOLD
    THR = (T / 4 * 2, T / 2 * 4, T * 8)

    nc = bass.Bass("TRN2", target_bir_lowering=False, debug=False,
                   num_devices=N_CORES)
    xsh = nc.dram_tensor("xs", [128, NPLANE, IMG, 258], F16, kind="ExternalInput")
    sth = nc.dram_tensor("st", [128, NSTAT * 128], F16, kind="ExternalInput")
    outh = nc.dram_tensor("res", [128, NACC], F32, kind="ExternalOutput")

    with tile.TileContext(nc) as tc, ExitStack() as ctx:
        v = nc.vector
        sc = nc.scalar

        pp = ctx.enter_context(tc.tile_pool(name="persist", bufs=1))
        xst = pp.tile([128, NPLANE, IMG, 258], F16, tag="xst")
        stats = pp.tile([128, NSTAT * 128], F16, tag="stats")
        out = pp.tile([128, 4, IMG, 512], F16, tag="out")     # [slot=t, m]
        Ee = pp.tile([128, 2, IMG, 512], F16, tag="Ee")       # [ph, m]
        Oo = pp.tile([128, 2, IMG, 512], F16, tag="Oo")
        sw = pp.tile([128, 2, IMG, 512], F16, tag="sw")       # [ph, m]
        dw = pp.tile([128, 2, IMG, 512], F16, tag="dw")
        ll1 = pp.tile([128, 2, IMG, 256], F16, tag="ll1")     # [th, m]
        det1 = pp.tile([128, 2, 3, 2, 2, 256], F16, tag="det1")  # [P, b, ph, mp]
        ll2 = pp.tile([128, IMG, 128], F16, tag="ll2")
        det2 = pp.tile([128, 3, IMG, 128], F16, tag="det2")
        det3 = pp.tile([128, 3, IMG, 64], F16, tag="det3")
        dtmp = pp.tile([128, 2, IMG, 258], F16, tag="dtmp")
        dlp = pp.tile([128, 2, IMG, 258], F16, tag="dlp")
        deadc = pp.tile([128, 2, 512], F16, tag="deadc")
        deadw = pp.tile([128, 2, 256], F16, tag="deadw")
        deadt = pp.tile([128, 3072], F16, tag="deadt")
        acc = pp.tile([128, NACC], F32, tag="acc")
        warm = pp.tile([128, 512], F16, tag="warm")

        ppre = ctx.enter_context(tc.tile_pool(name="ppre", bufs=2, space="PSUM"))
        pc = ctx.enter_context(tc.tile_pool(name="pc", bufs=2, space="PSUM"))

        # ---------------- input DMAs (conv-critical first) ----------------
        nc.sync.dma_start(out=stats[:, 0:32 * 128], in_=sth.ap()[:, 0:32 * 128])
        nc.sync.dma_start(out=xst[:, 0:4, :, :], in_=xsh.ap()[:, 0:4])

        v.memset(warm[:, :], 0.0)
        v.memset(acc[:, :], 0.0)

        # PE warm-up while inputs land
        wps = ppre.tile([128, 2, 512], F32, name="wps", tag="pre")
        for _ in range(7):
            nc.tensor.matmul(wps[:, 0, :], warm[:, 0:128], warm[:, :],
                             start=True, stop=True)

        # ---------------- conv units ----------------
        def conv_unit(m, half):
            pre = ppre.tile([128, 2, 512], F32, name="pre", tag="pre")
            for ti, t in enumerate(PAIRS[half]):
                mv = xst[:, MOVP[t], m, :]
                for par in range(2):
                    # edge-correction 1-col matmul folded into the group:
                    # par0 fixes fine row 0 (free slot 0), par1 row 511 (511)
                    nc.tensor.matmul(
                        pre[:, ti, 256 * par:256 * par + 256],
                        stats[:, st_conv(t, par, 0):st_conv(t, par, 0) + 128],
                        mv[:, 0:256], start=True, stop=False)
                    nc.tensor.matmul(
                        pre[:, ti, 256 * par:256 * par + 256],
                        stats[:, st_conv(t, par, 1):st_conv(t, par, 1) + 128],
                        mv[:, 1:257], start=False, stop=False)
                    if par == 0:
                        nc.tensor.matmul(
                            pre[:, ti, 0:1],
                            stats[:, st_corr(t, 0):st_corr(t, 0) + 128],
                            mv[:, 1:2], start=False, stop=False)
                    else:
                        nc.tensor.matmul(
                            pre[:, ti, 511:512],
                            stats[:, st_corr(t, 1):st_corr(t, 1) + 128],
                            mv[:, 256:257], start=False, stop=False)
                    nc.tensor.matmul(
                        pre[:, ti, 256 * par:256 * par + 256],
                        stats[:, st_conv(t, par, 2):st_conv(t, par, 2) + 128],
                        mv[:, 2:258], start=False, stop=True)
            if m in (1, 3):
                for ti, t in enumerate(PAIRS[half]):
                    v.tensor_scalar(out=out[:, t, m, :], in0=pre[:, ti, :],
                                    scalar1=0.0, scalar2=1.0,
                                    op0=Alu.max, op1=Alu.min)
            else:
                for ti, t in enumerate(PAIRS[half]):
                    sc.activation(out=out[:, t, m, :], in_=pre[:, ti, :],
                                  func=Act.Relu)

        def min_img(m):
            v.tensor_scalar(out=out[:, :, m, :], in0=out[:, :, m, :],
                            scalar1=1.0, scalar2=None, op0=Alu.min)

        # ---------------- c units ----------------
        def c_unit(m, half):
            ct = pc.tile([128, 2, 512], F32, name="ct", tag="ct")
            for ti, t in enumerate(PAIRS[half]):
                mv = xst[:, CPLANE[t], m, :]
                for par in range(2):
                    cols = slice(256 * par, 256 * par + 256)
                    for j, (dn, r) in enumerate(CTAPS[par]):
                        nc.tensor.matmul(ct[:, ti, cols],
                                         stats[:, st_c(t, par, j):st_c(t, par, j) + 128],
                                         mv[:, dn + 1:dn + 257],
                                         start=(j == 0), stop=False)
                    nc.tensor.matmul(ct[:, ti, cols],
                                     stats[:, ST_I:ST_I + 128],
                                     out[:, t, m, cols],
                                     start=False, stop=True)
            sc.activation(out=deadc[:, :, :], in_=ct[:, :, :], func=Act.Square,
                          accum_out=acc[:, 2 * m + half:2 * m + half + 1])

        # ---------------- delta term ----------------
        def delta_img(m):
            Wp = pc.tile([128, 2, 512], F32, name="wp", tag="ct")
            for ch in range(2):
                nc.tensor.matmul(Wp[:, ch, 0:256],
                                 stats[:, st_g(ch, 0):st_g(ch, 0) + 128],
                                 dlp[:, ch, m, 1:257], start=True, stop=False)
                nc.tensor.matmul(Wp[:, ch, 0:256],
                                 stats[:, st_g(ch, 1):st_g(ch, 1) + 128],
                                 dlp[:, 1 - ch, m, 1:257], start=False, stop=True)
            v.tensor_tensor(out=deadw[:, :, :], in0=xst[:, 8:10, m, 1:257],
                            in1=Wp[:, :, 0:256], op=Alu.mult)
            v.tensor_scalar(out=deadw[:, :, :], in0=deadw[:, :, :], scalar1=0.0,
                            scalar2=None, op0=Alu.bypass, op1=Alu.add,
                            accum_out=acc[:, 12 + m:13 + m])

        # ---------------- wavelet lvl1 ----------------
        outr = out[:, :, :, :].rearrange("p (sa sb) m f -> p sb sa (m f)",
                                         sa=2, sb=2)

        def shuffle_pair(P):
            fs = slice(1024 * P, 1024 * P + 1024)
            ms = slice(2 * P, 2 * P + 2)
            nc.sync.dma_start(out=Ee[0:64, :, ms, :], in_=outr[0:64, 0, :, fs])
            nc.sync.dma_start(out=Ee[64:128, :, ms, :], in_=outr[0:64, 1, :, fs])
            nc.sync.dma_start(out=Oo[0:64, :, ms, :], in_=outr[64:128, 0, :, fs])
            nc.sync.dma_start(out=Oo[64:128, :, ms, :], in_=outr[64:128, 1, :, fs])

        def lvl1_tt(P):
            ms = slice(2 * P, 2 * P + 2)
            eer = Ee[:, :, ms, :]
            oor = Oo[:, :, ms, :]
            v.tensor_tensor(out=sw[:, :, ms, :], in0=eer, in1=oor, op=Alu.add)
            v.tensor_tensor(out=dw[:, :, ms, :], in0=eer, in1=oor, op=Alu.subtract)
            v.tensor_tensor(out=ll1[:, :, ms, :], in0=sw[:, :, ms, 0:256],
                            in1=sw[:, :, ms, 256:512], op=Alu.add)
            v.tensor_tensor(out=det1[:, P, 0, :, :, :], in0=dw[:, :, ms, 0:256],
                            in1=dw[:, :, ms, 256:512], op=Alu.add)
            v.tensor_tensor(out=det1[:, P, 1, :, :, :], in0=sw[:, :, ms, 0:256],
                            in1=sw[:, :, ms, 256:512], op=Alu.subtract)
            v.tensor_tensor(out=det1[:, P, 2, :, :, :], in0=dw[:, :, ms, 0:256],
                            in1=dw[:, :, ms, 256:512], op=Alu.subtract)

        def lvl1_ts(P):
            flat = det1[:, P, :, :, :, :].rearrange("p b ph m f -> p (b ph m f)")
            sc.activation(out=flat, in_=flat, func=Act.Abs)
            v.tensor_scalar(out=deadt[:, 0:3072], in0=flat, scalar1=THR[0],
                            scalar2=None, op0=Alu.min, op1=Alu.add,
                            accum_out=acc[:, 8 + P:9 + P])

        # ---------------- wavelet lvl2/lvl3 on PE ----------------
        def l1v(th, rp):
            return ll1[:, th, :, :].rearrange(
                "p m (k two) -> p m k two", two=2)[..., rp]

        def lvl2():
            pa = ppre.tile([128, 2, 512], F32, name="pa", tag="pre")
            pb = pc.tile([128, 2, 512], F32, name="pb", tag="ct")
            bands = ((pa, 0, 0, (0, 0)), (pa, 1, 1, (0, 0)),
                     (pb, 0, 0, (0, 1)), (pb, 1, 1, (0, 1)))
            # (tile, slot, sd, rp-neg pattern): band0=ll2(sum,+,+),
            # band1=det2[0](dif,+,+), band2=det2[1](sum,+,-), band3=det2[2](dif,+,-)
            for bi, (tile_, slot, sd, negs) in enumerate(bands):
                dstv = tile_[:, slot, :].rearrange("p (m k) -> p m k", m=IMG)
                k = 0
                for rp in range(2):
                    for th in range(2):
                        nc.tensor.matmul(
                            dstv, stats[:, st_l2(sd, th, negs[rp]):
                                        st_l2(sd, th, negs[rp]) + 128],
                            l1v(th, rp), start=(k == 0), stop=(k == 3))
                        k += 1
            sc.activation(out=ll2[:, :, :],
                          in_=pa[:, 0, :].rearrange("p (m k) -> p m k", m=IMG),
                          func=Act.Copy)
            sc.activation(out=det2[:, 0, :, :],
                          in_=pa[:, 1, :].rearrange("p (m k) -> p m k", m=IMG),
                          func=Act.Abs)
            sc.activation(out=det2[:, 1, :, :],
                          in_=pb[:, 0, :].rearrange("p (m k) -> p m k", m=IMG),
                          func=Act.Abs)
            sc.activation(out=det2[:, 2, :, :],
                          in_=pb[:, 1, :].rearrange("p (m k) -> p m k", m=IMG),
                          func=Act.Abs)
            flat = det2[:, :, :, :].rearrange("p b m f -> p (b m f)")
            v.tensor_scalar(out=deadt[:, 0:1536], in0=flat, scalar1=THR[1],
                            scalar2=None, op0=Alu.min, op1=Alu.add,
                            accum_out=acc[:, 10:11])

        def l2v(rp):
            return ll2[:, :, :].rearrange(
                "p m (k two) -> p m k two", two=2)[..., rp]

        def lvl3():
            p3 = ppre.tile([128, 2, 512], F32, name="p3", tag="pre")
            regions = (p3[:, 0, 0:256], p3[:, 0, 256:512], p3[:, 1, 0:256])
            for b in range(3):
                sd = (1, 0, 1)[b]
                negs = ((0, 0), (0, 1), (0, 1))[b]
                dstv = regions[b].rearrange("p (m k) -> p m k", m=IMG)
                for rp in range(2):
                    nc.tensor.matmul(
                        dstv, stats[:, st_l3(sd, negs[rp]):
                                    st_l3(sd, negs[rp]) + 128],
                        l2v(rp), start=(rp == 0), stop=(rp == 1))
                sc.activation(out=det3[:, b, :, :], in_=dstv, func=Act.Abs)
            flat = det3[:, :, :, :].rearrange("p b m f -> p (b m f)")
            v.tensor_scalar(out=deadt[:, 0:768], in0=flat, scalar1=THR[2],
                            scalar2=None, op0=Alu.min, op1=Alu.add,
                            accum_out=acc[:, 11:12])

        # ---------------- main schedule ----------------
        for m in range(IMG):
            conv_unit(m, 0)
            if m == 0:
                sc.dma_start(out=xst[:, 4:8, :, :], in_=xsh.ap()[:, 4:8])
            conv_unit(m, 1)
            if m in (0, 2):
                min_img(m)
            if m == 0:
                sc.dma_start(out=xst[:, 8:10, :, :], in_=xsh.ap()[:, 8:10])
                sc.dma_start(out=stats[:, 32 * 128:],
                             in_=sth.ap()[:, 32 * 128:])
            if m == 1:
                shuffle_pair(0)
                lvl1_tt(0)
            if m == 2:
                # delta' = Gv/1.25 row factor: d + 0.3*(d shifted +-1)
                v.tensor_scalar(out=dtmp[:, :, :, :], in0=xst[:, 8:10, :, :],
                                scalar1=0.3, scalar2=None, op0=Alu.mult)
                v.tensor_tensor(out=dlp[:, :, :, 1:257],
                                in0=dtmp[:, :, :, 0:256],
                                in1=xst[:, 8:10, :, 1:257], op=Alu.add)
                v.tensor_tensor(out=dlp[:, :, :, 1:257],
                                in0=dtmp[:, :, :, 2:258],
                                in1=dlp[:, :, :, 1:257], op=Alu.add)
                lvl1_ts(0)
            if m == 3:
                shuffle_pair(1)
                lvl1_tt(1)
                lvl1_ts(1)
        for m in range(IMG):
            c_unit(m, 0)
            c_unit(m, 1)
        lvl2()
        lvl3()
        for m in range(IMG):
            delta_img(m)

        nc.sync.dma_start(out=outh.ap(), in_=acc[:, :])

    import os
    if not os.environ.get("SKIP_WAIT_SPLIT"):
        _split_multiwaits(nc, mybir)
    return nc


def _split_multiwaits(nc, mybir):
    """HW instructions support exactly ONE sync-wait; split extras into
    standalone Drains."""
    for f in nc.m.functions:
        for bb in f.blocks:
            i = 0
            while i < len(bb.instructions):
                ins = bb.instructions[i]
                si = getattr(ins, "sync_info", None)
                if si is not None and si.on_wait and len(si.on_wait) > 1:
                    waits = list(si.on_wait)
                    for w in waits[:-1]:
                        d = mybir.InstDrain(
                            name=nc.get_next_instruction_name(),
                            ins=[], outs=[], bass_is_fusable=False)
                        d.engine = ins.engine
                        d.sync_info = mybir.SyncInfo(on_wait=[w], on_update=[])
                        bb.instructions.insert(i, d)
                        i += 1
                    ins.sync_info = mybir.SyncInfo(
                        on_wait=[waits[-1]], on_update=list(si.on_update))
                i += 1


def _get_nc():
    if "nc" not in _CACHE:
        _CACHE["nc"] = _build()
    return _CACHE["nc"]


def make_in_maps(noisy_input, weight):
    x = np.asarray(noisy_input, dtype=np.float32).reshape(B_TOTAL, H, W)
    stats = _build_stats(weight)
    maps = []
    for c in range(N_CORES):
        xs = np.zeros((128, NPLANE, IMG, 258), dtype=np.float16)
        for m in range(IMG):
            img = x[c * IMG + m]
            p0 = img[0::2, 0::2].astype(np.float64)
            p3 = img[1::2, 1::2].astype(np.float64)
            qc = 2.0 * p0 + p3
            dl = p0 - p3
            for base, ph, wins in ((0, p0, True), (4, qc, True),
                                   (8, dl, False)):
                pt = np.ascontiguousarray(ph.T)          # [col, row]
                st = np.concatenate([pt[:, :1], pt, pt[:, -1:]], axis=1)
                xs[:, base + 0, m, :] = st[0:128]
                xs[:, base + 1, m, :] = st[128:256]
                if wins:
                    xs[:, base + 2, m, :] = st[62:190]   # Wa
                    xs[:, base + 3, m, :] = st[126:254]  # Wb
        maps.append({"xs": xs, "st": stats})
    return maps


def _host_combine(parts):
    S_c = S_d = 0.0
    wav = np.zeros(3)
    for p in parts:
        q = p.astype(np.float64)
        S_c += q[:, 0:8].sum()
        wav[0] += q[:, 8:10].sum()
        wav[1] += q[:, 10].sum()
        wav[2] += q[:, 11].sum()
        S_d += q[:, 12:16].sum()
    N = B_TOTAL * H * W
    n2n = (3.0 * S_c + (2.0 / 3.0) * S_d) / N
    wtot = 0.0
    for j in (1, 2, 3):
        lvl = 3 - j + 1
        Nj = B_TOTAL * (H // 2 ** j) ** 2 * 3
        wtot += (1.0 / lvl) * (wav[j - 1] / (2.0 ** j)) / Nj
    return np.float32(n2n + WAVELET_WEIGHT * wtot)


def kernel(noisy_input, weight):
    from concourse.bass_utils import run_bass_kernel_spmd
    nc = _get_nc()
    in_maps = make_in_maps(noisy_input, weight)
    res = run_bass_kernel_spmd(nc, in_maps, list(range(N_CORES)))
    return _host_combine([r["res"] for r in res.results])


# revision 28
# speedup vs baseline: 1.1910x; 1.0045x over previous
"""Trainium2 Bass kernel for nn_CombinedN2NWaveletLoss — v2 redesign.

Loss algebra: rec + 2*reg = 3*mean((out-c)^2) + (2/3)*mean((U delta U^T)^2)
with c = U qc U^T / 3, qc = 2*p0 + p3, delta = p0 - p3.  The delta term is
computed on the coarse grid via the Gram quadratic form sum(delta' * (G d G))
with G = U^T U (tridiagonal: diag 1.25, off 0.375, clamped edges); the row
factor (Gv/1.25, exact at the clamped edges thanks to dup guards) is staged
from the host, the column factor is 2 main + 2 halo PE matmuls per image.

Layout: transposed; partitions carry image COLUMNS, free dim carries rows.
The vertical (row) 2x upsample is folded into the PE stationaries as 3
coarse row-taps (T[par][dn] = sum_dy rho[par,dy,dn] * (K_dy @ U)), so the
conv consumes raw p0 directly.  Spurious conv zero-pad terms at fine rows
0/511 are removed by an extra 1-column matmul inside each accumulation
group.  Eviction: ACT Relu (psum->fp16) + one DVE min(.,1) per image at 4x.
The c field uses 2 row-taps (1/3 folded in) plus -I matmuls on clipped out,
evicted with ACT Square+accum.  Wavelet lvl1: batched shuffle DMAs + DVE
butterflies + ACT Abs + DVE min/accum.  lvl2/lvl3: entirely on PE via
(P_E +- P_O) permutation-butterfly stationaries with row-pair accumulation,
ACT Abs-fused psum evictions, DVE min/accum.
"""

import numpy as np

B_TOTAL = 32
N_CORES = 8
IMG = 4
H = W = 512
HC = 256
THRESHOLD = 50.0 / 255.0
GAMMA = 2.0
WAVELET_WEIGHT = 0.05
WIN = (0, 62, 126, 128)          # stationary col-window start per chunk t
PAIRS = ((0, 3), (1, 2))         # chunk pairs per unit half
MOVP = (0, 2, 3, 1)              # p0 moving plane per chunk t
CPLANE = (4, 6, 7, 5)            # qc moving plane per chunk t
NPLANE = 10                      # p0 b0,b1,Wa,Wb; qc b0,b1,Wa,Wb; d b0,b1
NACC = 16                        # 0..7 c-units, 8..9 lvl1 pairs, 10 lvl2, 11 lvl3, 12..15 delta
CTAPS = (((-1, 0.25), (0, 0.75)), ((0, 0.75), (1, 0.25)))

# stationary pack indices; conv+corr blocks grouped by unit pair so the
# first DMA covers unit half 0 (chunks t0, t3)
_PPOS = (0, 2, 3, 1)      # t -> position in pack


def st_conv(t, par, dni):
    return (_PPOS[t] * 8 + par * 3 + dni) * 128

def st_corr(t, e):
    return (_PPOS[t] * 8 + 6 + e) * 128

def st_c(t, par, j):
    # Tc[0][0]=Tc[1][1]=(0.25/3)U-window, Tc[0][1]=Tc[1][0]=(0.75/3)U-window:
    # only 2 distinct blocks per chunk.
    big = 1 if (par == 0) == (j == 1) else 0
    return (32 + t * 2 + big) * 128

ST_I = 40 * 128

def st_g(ch, h):
    return (41 + ch * 2 + h) * 128

def st_l2(sd, th, neg):              # sd: 0=sum(P_E+P_O) 1=dif; neg: rp sign
    return (45 + sd * 2 + th + 4 * neg) * 128

def st_l3(sd, neg):
    return (53 + sd + 2 * neg) * 128

NSTAT = 57

_CACHE = {}


def _brev6(q):
    return int(f"{q:06b}"[::-1], 2)


def _lanes(t):
    idx = np.empty(128, dtype=np.int64)
    for q in range(128):
        jp = 64 * t + _brev6(q % 64)
        idx[q] = 2 * jp + (0 if q < 64 else 1)
    return idx


def _upsample_matrix():
    U = np.zeros((H, HC), dtype=np.float64)
    for j in range(H):
        src = (j + 0.5) / 2.0 - 0.5
        k0 = int(np.floor(src))
        frac = src - k0
        for k, wgt in ((k0, 1 - frac), (k0 + 1, frac)):
            U[j, min(max(k, 0), HC - 1)] += wgt
    return U


def _conv_colops(w):
    U = _upsample_matrix()
    wm = np.asarray(w, dtype=np.float64).reshape(3, 3)
    Nm = []
    for dy in (-1, 0, 1):
        K = np.zeros((H, H))
        for j in range(H):
            for dx in (-1, 0, 1):
                if 0 <= j + dx < H:
                    K[j, j + dx] = wm[dy + 1, dx + 1]
        Nm.append(K @ U)
    return Nm


def _perm_l2(eo):
    """P with P[src_lane, dst] = 1: E2/O2[32tv+s] <- ll1[(tv&1)*64+32*eo+s]."""
    P = np.zeros((128, 128))
    for tv in range(4):
        for s in range(32):
            P[(tv & 1) * 64 + 32 * eo + s, 32 * tv + s] = 1.0
    return P


def _perm_l2_th(eo, th):
    """th-masked: only tv with tv>>1 == th contribute."""
    P = np.zeros((128, 128))
    for tv in range(4):
        if (tv >> 1) != th:
            continue
        for s in range(32):
            P[(tv & 1) * 64 + 32 * eo + s, 32 * tv + s] = 1.0
    return P


def _perm_l3(eo):
    P = np.zeros((128, 128))
    for tv in range(4):
        for s in range(16):
            P[32 * tv + 16 * eo + s, 16 * tv + s] = 1.0
    return P


def _build_stats(w):
    Nm = _conv_colops(w)
    rho = np.zeros((2, 3, 3))
    rho[0, 0, 0] = 0.75; rho[0, 0, 1] = 0.25
    rho[0, 1, 0] = 0.25; rho[0, 1, 1] = 0.75
    rho[0, 2, 1] = 0.75; rho[0, 2, 2] = 0.25
    rho[1, 0, 0] = 0.25; rho[1, 0, 1] = 0.75
    rho[1, 1, 1] = 0.75; rho[1, 1, 2] = 0.25
    rho[1, 2, 1] = 0.25; rho[1, 2, 2] = 0.75
    T = [[sum(rho[par, dyi, dni] * Nm[dyi] for dyi in range(3))
          for dni in range(3)] for par in range(2)]
    U = _upsample_matrix()
    G125 = 1.25 * (U.T @ U)

    out = np.zeros((128, NSTAT * 128), dtype=np.float32)
    for t in range(4):
        J = _lanes(t)
        for par in range(2):
            for dni in range(3):
                k = st_conv(t, par, dni)
                out[:, k:k + 128] = T[par][dni][J, WIN[t]:WIN[t] + 128].T
        for big, r in ((0, 0.25), (1, 0.75)):
            k = (32 + t * 2 + big) * 128
            out[:, k:k + 128] = (r / 3.0) * U[J, WIN[t]:WIN[t] + 128].T
        # conv edge corrections (zero-pad spurious terms), 1-col matmuls
        for e, Ncor in ((0, Nm[0]), (1, Nm[2])):
            k = st_corr(t, e)
            out[:, k:k + 128] = -Ncor[J, WIN[t]:WIN[t] + 128].T
    out[:, ST_I:ST_I + 128] = -np.eye(128)
    for ch in range(2):
        a = 128 * ch
        b = 128 * (1 - ch)
        out[:, st_g(ch, 0):st_g(ch, 0) + 128] = G125[a:a + 128, a:a + 128]
        out[:, st_g(ch, 1):st_g(ch, 1) + 128] = G125[b:b + 128, a:a + 128]
    # lvl2/lvl3 butterfly stationaries
    for th in range(2):
        S = _perm_l2_th(0, th)
        O = _perm_l2_th(1, th)
        for neg in range(2):
            sgn = -1.0 if neg else 1.0
            out[:, st_l2(0, th, neg):st_l2(0, th, neg) + 128] = sgn * (S + O)
            out[:, st_l2(1, th, neg):st_l2(1, th, neg) + 128] = sgn * (S - O)
    S3 = _perm_l3(0)
    O3 = _perm_l3(1)
    for neg in range(2):
        sgn = -1.0 if neg else 1.0
        out[:, st_l3(0, neg):st_l3(0, neg) + 128] = sgn * (S3 + O3)
        out[:, st_l3(1, neg):st_l3(1, neg) + 128] = sgn * (S3 - O3)
    return out.astype(np.float16)


def _build():
    import concourse.bass as bass
    import concourse.mybir as mybir
    import concourse.tile as tile
    from contextlib import ExitStack

    dt = mybir.dt
    Alu = mybir.AluOpType
    Act = mybir.ActivationFunctionType
    F16, F32 = dt.float16, dt.float32

    T = THRESHOLD
    THR = (T / 4 * 2, T / 2 * 4, T * 8)

    nc = bass.Bass("TRN2", target_bir_lowering=False, debug=False,
                   num_devices=N_CORES)
    xsh = nc.dram_tensor("xs", [128, NPLANE, IMG, 258], F16, kind="ExternalInput")
    sth = nc.dram_tensor("st", [128, NSTAT * 128], F16, kind="ExternalInput")
    outh = nc.dram_tensor("res", [128, NACC], F32, kind="ExternalOutput")

    with tile.TileContext(nc) as tc, ExitStack() as ctx:
        v = nc.vector
        sc = nc.scalar

        pp = ctx.enter_context(tc.tile_pool(name="persist", bufs=1))
        xst = pp.tile([128, NPLANE, IMG, 258], F16, tag="xst")
        stats = pp.tile([128, NSTAT * 128], F16, tag="stats")
        out = pp.tile([128, 4, IMG, 512], F16, tag="out")     # [slot=t, m]
        Ee = pp.tile([128, 2, IMG, 512], F16, tag="Ee")       # [ph, m]
        Oo = pp.tile([128, 2, IMG, 512], F16, tag="Oo")
        sw = pp.tile([128, 2, IMG, 512], F16, tag="sw")       # [ph, m]
        dw = pp.tile([128, 2, IMG, 512], F16, tag="dw")
        ll1 = pp.tile([128, 2, IMG, 256], F16, tag="ll1")     # [th, m]
        det1 = pp.tile([128, 2, 3, 2, 2, 256], F16, tag="det1")  # [P, b, ph, mp]
        ll2 = pp.tile([128, IMG, 128], F16, tag="ll2")
        det2 = pp.tile([128, 3, IMG, 128], F16, tag="det2")
        det3 = pp.tile([128, 3, IMG, 64], F16, tag="det3")
        dtmp = pp.tile([128, 2, IMG, 258], F16, tag="dtmp")
        dlp = pp.tile([128, 2, IMG, 258], F16, tag="dlp")
        deadc = pp.tile([128, 2, 512], F16, tag="deadc")
        deadw = pp.tile([128, 2, 256], F16, tag="deadw")
        deadt = pp.tile([128, 3072], F16, tag="deadt")
        acc = pp.tile([128, NACC], F32, tag="acc")
        warm = pp.tile([128, 512], F16, tag="warm")

        ppre = ctx.enter_context(tc.tile_pool(name="ppre", bufs=4, space="PSUM"))
        pc = ctx.enter_context(tc.tile_pool(name="pc", bufs=2, space="PSUM"))

        # -------- input DMAs: one FIFO queue, consumer order --------
        nc.sync.dma_start(out=stats[:, 0:16 * 128], in_=sth.ap()[:, 0:16 * 128])
        nc.sync.dma_start(out=xst[:, 0:2, :, :], in_=xsh.ap()[:, 0:2])
        nc.sync.dma_start(out=stats[:, 16 * 128:32 * 128],
                          in_=sth.ap()[:, 16 * 128:32 * 128])
        nc.sync.dma_start(out=xst[:, 2:4, :, :], in_=xsh.ap()[:, 2:4])
        nc.sync.dma_start(out=xst[:, 4:8, :, :], in_=xsh.ap()[:, 4:8])
        nc.sync.dma_start(out=xst[:, 8:10, :, :], in_=xsh.ap()[:, 8:10])
        nc.sync.dma_start(out=stats[:, 32 * 128:], in_=sth.ap()[:, 32 * 128:])

        v.memset(warm[:, :], 0.0)
        v.memset(acc[:, :], 0.0)

        # PE warm-up (p-state ramp) while inputs land
        wps = ppre.tile([128, 512], F32, name="wps", tag="pre")
        for _ in range(8):
            nc.tensor.matmul(wps[:, :], warm[:, 0:128], warm[:, :],
                             start=True, stop=True)

        # ---------------- conv units ----------------
        def conv_unit(m, half):
            pres = []
            for ti, t in enumerate(PAIRS[half]):
                pre = ppre.tile([128, 512], F32, name="pre", tag="pre")
                pres.append(pre)
                mv = xst[:, MOVP[t], m, :]
                for par in range(2):
                    # edge-correction 1-col matmul folded into the group:
                    # par0 fixes fine row 0 (free slot 0), par1 row 511 (511)
                    nc.tensor.matmul(
                        pre[:, 256 * par:256 * par + 256],
                        stats[:, st_conv(t, par, 0):st_conv(t, par, 0) + 128],
                        mv[:, 0:256], start=True, stop=False)
                    nc.tensor.matmul(
                        pre[:, 256 * par:256 * par + 256],
                        stats[:, st_conv(t, par, 1):st_conv(t, par, 1) + 128],
                        mv[:, 1:257], start=False, stop=False)
                    if par == 0:
                        nc.tensor.matmul(
                            pre[:, 0:1],
                            stats[:, st_corr(t, 0):st_corr(t, 0) + 128],
                            mv[:, 1:2], start=False, stop=False)
                    else:
                        nc.tensor.matmul(
                            pre[:, 511:512],
                            stats[:, st_corr(t, 1):st_corr(t, 1) + 128],
                            mv[:, 256:257], start=False, stop=False)
                    nc.tensor.matmul(
                        pre[:, 256 * par:256 * par + 256],
                        stats[:, st_conv(t, par, 2):st_conv(t, par, 2) + 128],
                        mv[:, 2:258], start=False, stop=True)
            if m in (1, 3):
                for ti, t in enumerate(PAIRS[half]):
                    v.tensor_scalar(out=out[:, t, m, :], in0=pres[ti][:, :],
                                    scalar1=0.0, scalar2=1.0,
                                    op0=Alu.max, op1=Alu.min)
            else:
                for ti, t in enumerate(PAIRS[half]):
                    sc.activation(out=out[:, t, m, :], in_=pres[ti][:, :],
                                  func=Act.Relu)

        def min_img(m):
            v.tensor_scalar(out=out[:, :, m, :], in0=out[:, :, m, :],
                            scalar1=1.0, scalar2=None, op0=Alu.min)

        # ---------------- c units ----------------
        def c_unit(m, half):
            ct = pc.tile([128, 2, 512], F32, name="ct", tag="ct")
            for ti, t in enumerate(PAIRS[half]):
                mv = xst[:, CPLANE[t], m, :]
                for par in range(2):
                    cols = slice(256 * par, 256 * par + 256)
                    for j, (dn, r) in enumerate(CTAPS[par]):
                        nc.tensor.matmul(ct[:, ti, cols],
                                         stats[:, st_c(t, par, j):st_c(t, par, j) + 128],
                                         mv[:, dn + 1:dn + 257],
                                         start=(j == 0), stop=False)
                    nc.tensor.matmul(ct[:, ti, cols],
                                     stats[:, ST_I:ST_I + 128],
                                     out[:, t, m, cols],
                                     start=False, stop=True)
            sc.activation(out=deadc[:, :, :], in_=ct[:, :, :], func=Act.Square,
                          accum_out=acc[:, 2 * m + half:2 * m + half + 1])

        # ---------------- delta term ----------------
        def delta_img(m):
            Wp = pc.tile([128, 2, 512], F32, name="wp", tag="ct")
            for ch in range(2):
                nc.tensor.matmul(Wp[:, ch, 0:256],
                                 stats[:, st_g(ch, 0):st_g(ch, 0) + 128],
                                 dlp[:, ch, m, 1:257], start=True, stop=False)
                nc.tensor.matmul(Wp[:, ch, 0:256],
                                 stats[:, st_g(ch, 1):st_g(ch, 1) + 128],
                                 dlp[:, 1 - ch, m, 1:257], start=False, stop=True)
            v.tensor_tensor(out=deadw[:, :, :], in0=xst[:, 8:10, m, 1:257],
                            in1=Wp[:, :, 0:256], op=Alu.mult)
            v.tensor_scalar(out=deadw[:, :, :], in0=deadw[:, :, :], scalar1=0.0,
                            scalar2=None, op0=Alu.bypass, op1=Alu.add,
                            accum_out=acc[:, 12 + m:13 + m])

        # ---------------- wavelet lvl1 ----------------
        outr = out[:, :, :, :].rearrange("p (sa sb) m f -> p sb sa (m f)",
                                         sa=2, sb=2)

        def shuffle_pair(P):
            fs = slice(1024 * P, 1024 * P + 1024)
            ms = slice(2 * P, 2 * P + 2)
            nc.sync.dma_start(out=Ee[0:64, :, ms, :], in_=outr[0:64, 0, :, fs])
            nc.sync.dma_start(out=Ee[64:128, :, ms, :], in_=outr[0:64, 1, :, fs])
            nc.sync.dma_start(out=Oo[0:64, :, ms, :], in_=outr[64:128, 0, :, fs])
            nc.sync.dma_start(out=Oo[64:128, :, ms, :], in_=outr[64:128, 1, :, fs])

        def lvl1_tt(P):
            ms = slice(2 * P, 2 * P + 2)
            eer = Ee[:, :, ms, :]
            oor = Oo[:, :, ms, :]
            v.tensor_tensor(out=sw[:, :, ms, :], in0=eer, in1=oor, op=Alu.add)
            v.tensor_tensor(out=dw[:, :, ms, :], in0=eer, in1=oor, op=Alu.subtract)
            v.tensor_tensor(out=ll1[:, :, ms, :], in0=sw[:, :, ms, 0:256],
                            in1=sw[:, :, ms, 256:512], op=Alu.add)
            v.tensor_tensor(out=det1[:, P, 0, :, :, :], in0=dw[:, :, ms, 0:256],
                            in1=dw[:, :, ms, 256:512], op=Alu.add)
            v.tensor_tensor(out=det1[:, P, 1, :, :, :], in0=sw[:, :, ms, 0:256],
                            in1=sw[:, :, ms, 256:512], op=Alu.subtract)
            v.tensor_tensor(out=det1[:, P, 2, :, :, :], in0=dw[:, :, ms, 0:256],
                            in1=dw[:, :, ms, 256:512], op=Alu.subtract)

        def lvl1_ts(P):
            flat = det1[:, P, :, :, :, :].rearrange("p b ph m f -> p (b ph m f)")
            sc.activation(out=flat, in_=flat, func=Act.Abs)
            v.tensor_scalar(out=deadt[:, 0:3072], in0=flat, scalar1=THR[0],
                            scalar2=None, op0=Alu.min, op1=Alu.add,
                            accum_out=acc[:, 8 + P:9 + P])

        # ---------------- wavelet lvl2/lvl3 on PE ----------------
        def l1v(th, rp):
            return ll1[:, th, :, :].rearrange(
                "p m (k two) -> p m k two", two=2)[..., rp]

        def lvl2():
            pa0 = ppre.tile([128, 512], F32, name="pa0", tag="pre")
            pa1 = ppre.tile([128, 512], F32, name="pa1", tag="pre")
            pb = pc.tile([128, 2, 512], F32, name="pb", tag="ct")
            bands = ((pa0, None, 0, (0, 0)), (pa1, None, 1, (0, 0)),
                     (pb, 0, 0, (0, 1)), (pb, 1, 1, (0, 1)))
            # (tile, slot, sd, rp-neg pattern): band0=ll2(sum,+,+),
            # band1=det2[0](dif,+,+), band2=det2[1](sum,+,-), band3=det2[2](dif,+,-)
            for bi, (tile_, slot, sd, negs) in enumerate(bands):
                base = tile_[:, :] if slot is None else tile_[:, slot, :]
                dstv = base.rearrange("p (m k) -> p m k", m=IMG)
                k = 0
                for rp in range(2):
                    for th in range(2):
                        nc.tensor.matmul(
                            dstv, stats[:, st_l2(sd, th, negs[rp]):
                                        st_l2(sd, th, negs[rp]) + 128],
                            l1v(th, rp), start=(k == 0), stop=(k == 3))
                        k += 1
            sc.activation(out=ll2[:, :, :],
                          in_=pa0[:, :].rearrange("p (m k) -> p m k", m=IMG),
                          func=Act.Copy)
            sc.activation(out=det2[:, 0, :, :],
                          in_=pa1[:, :].rearrange("p (m k) -> p m k", m=IMG),
                          func=Act.Abs)
            sc.activation(out=det2[:, 1, :, :],
                          in_=pb[:, 0, :].rearrange("p (m k) -> p m k", m=IMG),
                          func=Act.Abs)
            sc.activation(out=det2[:, 2, :, :],
                          in_=pb[:, 1, :].rearrange("p (m k) -> p m k", m=IMG),
                          func=Act.Abs)
            flat = det2[:, :, :, :].rearrange("p b m f -> p (b m f)")
            v.tensor_scalar(out=deadt[:, 0:1536], in0=flat, scalar1=THR[1],
                            scalar2=None, op0=Alu.min, op1=Alu.add,
                            accum_out=acc[:, 10:11])

        def l2v(rp):
            return ll2[:, :, :].rearrange(
                "p m (k two) -> p m k two", two=2)[..., rp]

        def lvl3():
            p3a = ppre.tile([128, 512], F32, name="p3a", tag="pre")
            p3b = ppre.tile([128, 512], F32, name="p3b", tag="pre")
            regions = (p3a[:, 0:256], p3a[:, 256:512], p3b[:, 0:256])
            for b in range(3):
                sd = (1, 0, 1)[b]
                negs = ((0, 0), (0, 1), (0, 1))[b]
                dstv = regions[b].rearrange("p (m k) -> p m k", m=IMG)
                for rp in range(2):
                    nc.tensor.matmul(
                        dstv, stats[:, st_l3(sd, negs[rp]):
                                    st_l3(sd, negs[rp]) + 128],
                        l2v(rp), start=(rp == 0), stop=(rp == 1))
                sc.activation(out=det3[:, b, :, :], in_=dstv, func=Act.Abs)
            flat = det3[:, :, :, :].rearrange("p b m f -> p (b m f)")
            v.tensor_scalar(out=deadt[:, 0:768], in0=flat, scalar1=THR[2],
                            scalar2=None, op0=Alu.min, op1=Alu.add,
                            accum_out=acc[:, 11:12])

        # ---------------- main schedule ----------------
        for m in range(IMG):
            conv_unit(m, 0)
            if m == 0:
                sc.dma_start(out=xst[:, 4:8, :, :], in_=xsh.ap()[:, 4:8])
            conv_unit(m, 1)
            if m in (0, 2):
                min_img(m)
            if m == 0:
                sc.dma_start(out=xst[:, 8:10, :, :], in_=xsh.ap()[:, 8:10])
                sc.dma_start(out=stats[:, 32 * 128:],
                             in_=sth.ap()[:, 32 * 128:])
            if m == 1:
                shuffle_pair(0)
                lvl1_tt(0)
            if m == 2:
                # delta' = Gv/1.25 row factor: d + 0.3*(d shifted +-1)
                v.tensor_scalar(out=dtmp[:, :, :, :], in0=xst[:, 8:10, :, :],
                                scalar1=0.3, scalar2=None, op0=Alu.mult)
                v.tensor_tensor(out=dlp[:, :, :, 1:257],
                                in0=dtmp[:, :, :, 0:256],
                                in1=xst[:, 8:10, :, 1:257], op=Alu.add)
                v.tensor_tensor(out=dlp[:, :, :, 1:257],
                                in0=dtmp[:, :, :, 2:258],
                                in1=dlp[:, :, :, 1:257], op=Alu.add)
                lvl1_ts(0)
            if m == 3:
                shuffle_pair(1)
                lvl1_tt(1)
                lvl1_ts(1)
        for m in range(IMG):
            c_unit(m, 0)
            c_unit(m, 1)
            delta_img(m)
        lvl2()
        lvl3()

        nc.sync.dma_start(out=outh.ap(), in_=acc[:, :])

    import os
    if not os.environ.get("SKIP_WAIT_SPLIT"):
        _split_multiwaits(nc, mybir)
    return nc


def _split_multiwaits(nc, mybir):
    """HW instructions support exactly ONE sync-wait; split extras into
    standalone Drains."""
    for f in nc.m.functions:
        for bb in f.blocks:
            i = 0
            while i < len(bb.instructions):
                ins = bb.instructions[i]
                si = getattr(ins, "sync_info", None)
                if si is not None and si.on_wait and len(si.on_wait) > 1:
                    waits = list(si.on_wait)
                    for w in waits[:-1]:
                        d = mybir.InstDrain(
                            name=nc.get_next_instruction_name(),
                            ins=[], outs=[], bass_is_fusable=False)
                        d.engine = ins.engine
                        d.sync_info = mybir.SyncInfo(on_wait=[w], on_update=[])
                        bb.instructions.insert(i, d)
                        i += 1
                    ins.sync_info = mybir.SyncInfo(
                        on_wait=[waits[-1]], on_update=list(si.on_update))
                i += 1


def _get_nc():
    if "nc" not in _CACHE:
        _CACHE["nc"] = _build()
    return _CACHE["nc"]


def make_in_maps(noisy_input, weight):
    x = np.asarray(noisy_input, dtype=np.float32).reshape(B_TOTAL, H, W)
    stats = _build_stats(weight)
    maps = []
    for c in range(N_CORES):
        xs = np.zeros((128, NPLANE, IMG, 258), dtype=np.float16)
        for m in range(IMG):
            img = x[c * IMG + m]
            p0 = img[0::2, 0::2].astype(np.float64)
            p3 = img[1::2, 1::2].astype(np.float64)
            qc = 2.0 * p0 + p3
            dl = p0 - p3
            for base, ph, wins in ((0, p0, True), (4, qc, True),
                                   (8, dl, False)):
                pt = np.ascontiguousarray(ph.T)          # [col, row]
                st = np.concatenate([pt[:, :1], pt, pt[:, -1:]], axis=1)
                xs[:, base + 0, m, :] = st[0:128]
                xs[:, base + 1, m, :] = st[128:256]
                if wins:
                    xs[:, base + 2, m, :] = st[62:190]   # Wa
                    xs[:, base + 3, m, :] = st[126:254]  # Wb
        maps.append({"xs": xs, "st": stats})
    return maps


def _host_combine(parts):
    S_c = S_d = 0.0
    wav = np.zeros(3)
    for p in parts:
        q = p.astype(np.float64)
        S_c += q[:, 0:8].sum()
        wav[0] += q[:, 8:10].sum()
        wav[1] += q[:, 10].sum()
        wav[2] += q[:, 11].sum()
        S_d += q[:, 12:16].sum()
    N = B_TOTAL * H * W
    n2n = (3.0 * S_c + (2.0 / 3.0) * S_d) / N
    wtot = 0.0
    for j in (1, 2, 3):
        lvl = 3 - j + 1
        Nj = B_TOTAL * (H // 2 ** j) ** 2 * 3
        wtot += (1.0 / lvl) * (wav[j - 1] / (2.0 ** j)) / Nj
    return np.float32(n2n + WAVELET_WEIGHT * wtot)


def kernel(noisy_input, weight):
    from concourse.bass_utils import run_bass_kernel_spmd
    nc = _get_nc()
    in_maps = make_in_maps(noisy_input, weight)
    res = run_bass_kernel_spmd(nc, in_maps, list(range(N_CORES)))
    return _host_combine([r["res"] for r in res.results])


# revision 29
# speedup vs baseline: 1.4533x; 1.2202x over previous
"""Trainium2 Bass kernel for nn_CombinedN2NWaveletLoss — v2 redesign.

Loss algebra: rec + 2*reg = 3*mean((out-c)^2) + (2/3)*mean((U delta U^T)^2)
with c = U qc U^T / 3, qc = 2*p0 + p3, delta = p0 - p3.  The delta term is
computed on the coarse grid via the Gram quadratic form sum(delta' * (G d G))
with G = U^T U (tridiagonal: diag 1.25, off 0.375, clamped edges); the row
factor (Gv/1.25, exact at the clamped edges thanks to dup guards) is staged
from the host, the column factor is 2 main + 2 halo PE matmuls per image.

Layout: transposed; partitions carry image COLUMNS, free dim carries rows.
The vertical (row) 2x upsample is folded into the PE stationaries as 3
coarse row-taps (T[par][dn] = sum_dy rho[par,dy,dn] * (K_dy @ U)), so the
conv consumes raw p0 directly.  Spurious conv zero-pad terms at fine rows
0/511 are removed by an extra 1-column matmul inside each accumulation
group.  Eviction: ACT Relu (psum->fp16) + one DVE min(.,1) per image at 4x.
The c field uses 2 row-taps (1/3 folded in) plus -I matmuls on clipped out,
evicted with ACT Square+accum.  Wavelet lvl1: batched shuffle DMAs + DVE
butterflies + ACT Abs + DVE min/accum.  lvl2/lvl3: entirely on PE via
(P_E +- P_O) permutation-butterfly stationaries with row-pair accumulation,
ACT Abs-fused psum evictions, DVE min/accum.
"""

import numpy as np

B_TOTAL = 32
N_CORES = 8
IMG = 4
H = W = 512
HC = 256
THRESHOLD = 50.0 / 255.0
GAMMA = 2.0
WAVELET_WEIGHT = 0.05
WIN = (0, 62, 126, 128)          # stationary col-window start per chunk t
PAIRS = ((0, 3), (1, 2))         # chunk pairs per unit half
MOVP = (0, 2, 3, 1)              # p0 moving plane per chunk t
CPLANE = (4, 6, 7, 5)            # qc moving plane per chunk t
NPLANE = 10                      # p0 b0,b1,Wa,Wb; qc b0,b1,Wa,Wb; d b0,b1
NACC = 16                        # 0..7 c-units, 8..9 lvl1 pairs, 10 lvl2, 11 lvl3, 12..15 delta
CTAPS = (((-1, 0.25), (0, 0.75)), ((0, 0.75), (1, 0.25)))

# stationary pack indices; conv+corr blocks grouped by unit pair so the
# first DMA covers unit half 0 (chunks t0, t3)
_PPOS = (0, 2, 3, 1)      # t -> position in pack


def st_conv(t, par, dni):
    return (_PPOS[t] * 8 + par * 3 + dni) * 128

def st_corr(t, e):
    return (_PPOS[t] * 8 + 6 + e) * 128

def st_c(t, par, j):
    # Tc[0][0]=Tc[1][1]=(0.25/3)U-window, Tc[0][1]=Tc[1][0]=(0.75/3)U-window:
    # only 2 distinct blocks per chunk.
    big = 1 if (par == 0) == (j == 1) else 0
    return (32 + t * 2 + big) * 128

ST_I = 40 * 128

def st_g(ch, h):
    return (41 + ch * 2 + h) * 128

def st_l2(sd, th, neg):              # sd: 0=sum(P_E+P_O) 1=dif; neg: rp sign
    return (45 + sd * 2 + th + 4 * neg) * 128

def st_l3(sd, neg):
    return (53 + sd + 2 * neg) * 128

NSTAT = 57

_CACHE = {}


def _brev6(q):
    return int(f"{q:06b}"[::-1], 2)


def _lanes(t):
    idx = np.empty(128, dtype=np.int64)
    for q in range(128):
        jp = 64 * t + _brev6(q % 64)
        idx[q] = 2 * jp + (0 if q < 64 else 1)
    return idx


def _upsample_matrix():
    U = np.zeros((H, HC), dtype=np.float64)
    for j in range(H):
        src = (j + 0.5) / 2.0 - 0.5
        k0 = int(np.floor(src))
        frac = src - k0
        for k, wgt in ((k0, 1 - frac), (k0 + 1, frac)):
            U[j, min(max(k, 0), HC - 1)] += wgt
    return U


def _conv_colops(w):
    U = _upsample_matrix()
    wm = np.asarray(w, dtype=np.float64).reshape(3, 3)
    Nm = []
    for dy in (-1, 0, 1):
        K = np.zeros((H, H))
        for j in range(H):
            for dx in (-1, 0, 1):
                if 0 <= j + dx < H:
                    K[j, j + dx] = wm[dy + 1, dx + 1]
        Nm.append(K @ U)
    return Nm


def _perm_l2(eo):
    """P with P[src_lane, dst] = 1: E2/O2[32tv+s] <- ll1[(tv&1)*64+32*eo+s]."""
    P = np.zeros((128, 128))
    for tv in range(4):
        for s in range(32):
            P[(tv & 1) * 64 + 32 * eo + s, 32 * tv + s] = 1.0
    return P


def _perm_l2_th(eo, th):
    """th-masked: only tv with tv>>1 == th contribute."""
    P = np.zeros((128, 128))
    for tv in range(4):
        if (tv >> 1) != th:
            continue
        for s in range(32):
            P[(tv & 1) * 64 + 32 * eo + s, 32 * tv + s] = 1.0
    return P


def _perm_l3(eo):
    P = np.zeros((128, 128))
    for tv in range(4):
        for s in range(16):
            P[32 * tv + 16 * eo + s, 16 * tv + s] = 1.0
    return P


def _build_stats(w):
    Nm = _conv_colops(w)
    rho = np.zeros((2, 3, 3))
    rho[0, 0, 0] = 0.75; rho[0, 0, 1] = 0.25
    rho[0, 1, 0] = 0.25; rho[0, 1, 1] = 0.75
    rho[0, 2, 1] = 0.75; rho[0, 2, 2] = 0.25
    rho[1, 0, 0] = 0.25; rho[1, 0, 1] = 0.75
    rho[1, 1, 1] = 0.75; rho[1, 1, 2] = 0.25
    rho[1, 2, 1] = 0.25; rho[1, 2, 2] = 0.75
    T = [[sum(rho[par, dyi, dni] * Nm[dyi] for dyi in range(3))
          for dni in range(3)] for par in range(2)]
    U = _upsample_matrix()
    G125 = 1.25 * (U.T @ U)

    out = np.zeros((128, NSTAT * 128), dtype=np.float32)
    for t in range(4):
        J = _lanes(t)
        for par in range(2):
            for dni in range(3):
                k = st_conv(t, par, dni)
                out[:, k:k + 128] = T[par][dni][J, WIN[t]:WIN[t] + 128].T
        for big, r in ((0, 0.25), (1, 0.75)):
            k = (32 + t * 2 + big) * 128
            out[:, k:k + 128] = (r / 3.0) * U[J, WIN[t]:WIN[t] + 128].T
        # conv edge corrections (zero-pad spurious terms), 1-col matmuls
        for e, Ncor in ((0, Nm[0]), (1, Nm[2])):
            k = st_corr(t, e)
            out[:, k:k + 128] = -Ncor[J, WIN[t]:WIN[t] + 128].T
    out[:, ST_I:ST_I + 128] = -np.eye(128)
    for ch in range(2):
        a = 128 * ch
        b = 128 * (1 - ch)
        out[:, st_g(ch, 0):st_g(ch, 0) + 128] = G125[a:a + 128, a:a + 128]
        out[:, st_g(ch, 1):st_g(ch, 1) + 128] = G125[b:b + 128, a:a + 128]
    # lvl2/lvl3 butterfly stationaries
    for th in range(2):
        S = _perm_l2_th(0, th)
        O = _perm_l2_th(1, th)
        for neg in range(2):
            sgn = -1.0 if neg else 1.0
            out[:, st_l2(0, th, neg):st_l2(0, th, neg) + 128] = sgn * (S + O)
            out[:, st_l2(1, th, neg):st_l2(1, th, neg) + 128] = sgn * (S - O)
    S3 = _perm_l3(0)
    O3 = _perm_l3(1)
    for neg in range(2):
        sgn = -1.0 if neg else 1.0
        out[:, st_l3(0, neg):st_l3(0, neg) + 128] = sgn * (S3 + O3)
        out[:, st_l3(1, neg):st_l3(1, neg) + 128] = sgn * (S3 - O3)
    return out.astype(np.float16)


def _build():
    import concourse.bass as bass
    import concourse.mybir as mybir
    import concourse.tile as tile
    from contextlib import ExitStack

    dt = mybir.dt
    Alu = mybir.AluOpType
    Act = mybir.ActivationFunctionType
    F16, F32 = dt.float16, dt.float32

    T = THRESHOLD
    THR = (T / 4 * 2, T / 2 * 4, T * 8)

    nc = bass.Bass("TRN2", target_bir_lowering=False, debug=False,
                   num_devices=N_CORES)
    xsh = nc.dram_tensor("xs", [128, NPLANE, IMG, 258], F16, kind="ExternalInput")
    sth = nc.dram_tensor("st", [128, NSTAT * 128], F16, kind="ExternalInput")
    outh = nc.dram_tensor("res", [128, NACC], F32, kind="ExternalOutput")

    with tile.TileContext(nc) as tc, ExitStack() as ctx:
        v = nc.vector
        sc = nc.scalar

        pp = ctx.enter_context(tc.tile_pool(name="persist", bufs=1))
        xst = pp.tile([128, NPLANE, IMG, 258], F16, tag="xst")
        stats = pp.tile([128, NSTAT * 128], F16, tag="stats")
        out = pp.tile([128, 4, IMG, 512], F16, tag="out")     # [slot=t, m]
        Ee = pp.tile([128, 2, IMG, 512], F16, tag="Ee")       # [ph, m]
        Oo = pp.tile([128, 2, IMG, 512], F16, tag="Oo")
        sw = pp.tile([128, 2, IMG, 512], F16, tag="sw")       # [ph, m]
        dw = pp.tile([128, 2, IMG, 512], F16, tag="dw")
        ll1 = pp.tile([128, 2, IMG, 256], F16, tag="ll1")     # [th, m]
        det1 = pp.tile([128, 2, 3, 2, 2, 256], F16, tag="det1")  # [P, b, ph, mp]
        ll2 = pp.tile([128, IMG, 128], F16, tag="ll2")
        det2 = pp.tile([128, 3, IMG, 128], F16, tag="det2")
        det3 = pp.tile([128, 3, IMG, 64], F16, tag="det3")
        dtmp = pp.tile([128, 2, IMG, 258], F16, tag="dtmp")
        dlp = pp.tile([128, 2, IMG, 258], F16, tag="dlp")
        deadc = pp.tile([128, 2, 512], F16, tag="deadc")
        deadw = pp.tile([128, 2, 256], F16, tag="deadw")
        deadt = pp.tile([128, 3072], F16, tag="deadt")
        acc = pp.tile([128, NACC], F32, tag="acc")
        warm = pp.tile([128, 512], F16, tag="warm")

        ppre = ctx.enter_context(tc.tile_pool(name="ppre", bufs=4, space="PSUM"))
        pc = ctx.enter_context(tc.tile_pool(name="pc", bufs=2, space="PSUM"))

        # -------- input DMAs: one FIFO queue, consumer order --------
        nc.sync.dma_start(out=stats[:, 0:16 * 128], in_=sth.ap()[:, 0:16 * 128])
        nc.sync.dma_start(out=xst[:, 0:2, :, :], in_=xsh.ap()[:, 0:2])
        nc.sync.dma_start(out=stats[:, 16 * 128:32 * 128],
                          in_=sth.ap()[:, 16 * 128:32 * 128])
        nc.sync.dma_start(out=xst[:, 2:4, :, :], in_=xsh.ap()[:, 2:4])
        nc.sync.dma_start(out=xst[:, 4:8, :, :], in_=xsh.ap()[:, 4:8])
        nc.sync.dma_start(out=xst[:, 8:10, :, :], in_=xsh.ap()[:, 8:10])
        nc.sync.dma_start(out=stats[:, 32 * 128:], in_=sth.ap()[:, 32 * 128:])

        v.memset(warm[:, :], 0.0)
        v.memset(acc[:, :], 0.0)

        # PE warm-up (p-state ramp) while inputs land
        wps = ppre.tile([128, 512], F32, name="wps", tag="pre")
        for _ in range(8):
            nc.tensor.matmul(wps[:, :], warm[:, 0:128], warm[:, :],
                             start=True, stop=True)

        # ---------------- conv units ----------------
        def conv_unit(m, half):
            pres = []
            for ti, t in enumerate(PAIRS[half]):
                pre = ppre.tile([128, 512], F32, name="pre", tag="pre")
                pres.append(pre)
                mv = xst[:, MOVP[t], m, :]
                for par in range(2):
                    # edge-correction 1-col matmul folded into the group:
                    # par0 fixes fine row 0 (free slot 0), par1 row 511 (511)
                    nc.tensor.matmul(
                        pre[:, 256 * par:256 * par + 256],
                        stats[:, st_conv(t, par, 0):st_conv(t, par, 0) + 128],
                        mv[:, 0:256], start=True, stop=False)
                    nc.tensor.matmul(
                        pre[:, 256 * par:256 * par + 256],
                        stats[:, st_conv(t, par, 1):st_conv(t, par, 1) + 128],
                        mv[:, 1:257], start=False, stop=False)
                    if par == 0:
                        nc.tensor.matmul(
                            pre[:, 0:1],
                            stats[:, st_corr(t, 0):st_corr(t, 0) + 128],
                            mv[:, 1:2], start=False, stop=False)
                    else:
                        nc.tensor.matmul(
                            pre[:, 511:512],
                            stats[:, st_corr(t, 1):st_corr(t, 1) + 128],
                            mv[:, 256:257], start=False, stop=False)
                    nc.tensor.matmul(
                        pre[:, 256 * par:256 * par + 256],
                        stats[:, st_conv(t, par, 2):st_conv(t, par, 2) + 128],
                        mv[:, 2:258], start=False, stop=True)
            if m in (1, 3):
                for ti, t in enumerate(PAIRS[half]):
                    v.tensor_scalar(out=out[:, t, m, :], in0=pres[ti][:, :],
                                    scalar1=0.0, scalar2=1.0,
                                    op0=Alu.max, op1=Alu.min)
            else:
                for ti, t in enumerate(PAIRS[half]):
                    sc.activation(out=out[:, t, m, :], in_=pres[ti][:, :],
                                  func=Act.Relu)

        def min_img(m):
            v.tensor_scalar(out=out[:, :, m, :], in0=out[:, :, m, :],
                            scalar1=1.0, scalar2=None, op0=Alu.min)

        # ---------------- c units ----------------
        def c_unit(m, half):
            ct = pc.tile([128, 2, 512], F32, name="ct", tag="ct")
            for ti, t in enumerate(PAIRS[half]):
                mv = xst[:, CPLANE[t], m, :]
                for par in range(2):
                    cols = slice(256 * par, 256 * par + 256)
                    for j, (dn, r) in enumerate(CTAPS[par]):
                        nc.tensor.matmul(ct[:, ti, cols],
                                         stats[:, st_c(t, par, j):st_c(t, par, j) + 128],
                                         mv[:, dn + 1:dn + 257],
                                         start=(j == 0), stop=False)
                    nc.tensor.matmul(ct[:, ti, cols],
                                     stats[:, ST_I:ST_I + 128],
                                     out[:, t, m, cols],
                                     start=False, stop=True)
            sc.activation(out=deadc[:, :, :], in_=ct[:, :, :], func=Act.Square,
                          accum_out=acc[:, 2 * m + half:2 * m + half + 1])

        # ---------------- delta term ----------------
        def delta_img(m):
            Wp = pc.tile([128, 2, 512], F32, name="wp", tag="ct")
            for ch in range(2):
                nc.tensor.matmul(Wp[:, ch, 0:256],
                                 stats[:, st_g(ch, 0):st_g(ch, 0) + 128],
                                 dlp[:, ch, m, 1:257], start=True, stop=False)
                nc.tensor.matmul(Wp[:, ch, 0:256],
                                 stats[:, st_g(ch, 1):st_g(ch, 1) + 128],
                                 dlp[:, 1 - ch, m, 1:257], start=False, stop=True)
            v.tensor_tensor(out=deadw[:, :, :], in0=xst[:, 8:10, m, 1:257],
                            in1=Wp[:, :, 0:256], op=Alu.mult)
            v.tensor_scalar(out=deadw[:, :, :], in0=deadw[:, :, :], scalar1=0.0,
                            scalar2=None, op0=Alu.bypass, op1=Alu.add,
                            accum_out=acc[:, 12 + m:13 + m])

        # ---------------- wavelet lvl1 ----------------
        outr = out[:, :, :, :].rearrange("p (sa sb) m f -> p sb sa (m f)",
                                         sa=2, sb=2)

        def shuffle_pair(P):
            fs = slice(1024 * P, 1024 * P + 1024)
            ms = slice(2 * P, 2 * P + 2)
            nc.sync.dma_start(out=Ee[0:64, :, ms, :], in_=outr[0:64, 0, :, fs])
            nc.sync.dma_start(out=Ee[64:128, :, ms, :], in_=outr[0:64, 1, :, fs])
            nc.sync.dma_start(out=Oo[0:64, :, ms, :], in_=outr[64:128, 0, :, fs])
            nc.sync.dma_start(out=Oo[64:128, :, ms, :], in_=outr[64:128, 1, :, fs])

        def lvl1_tt(P):
            ms = slice(2 * P, 2 * P + 2)
            eer = Ee[:, :, ms, :]
            oor = Oo[:, :, ms, :]
            v.tensor_tensor(out=sw[:, :, ms, :], in0=eer, in1=oor, op=Alu.add)
            v.tensor_tensor(out=dw[:, :, ms, :], in0=eer, in1=oor, op=Alu.subtract)
            v.tensor_tensor(out=ll1[:, :, ms, :], in0=sw[:, :, ms, 0:256],
                            in1=sw[:, :, ms, 256:512], op=Alu.add)
            v.tensor_tensor(out=det1[:, P, 0, :, :, :], in0=dw[:, :, ms, 0:256],
                            in1=dw[:, :, ms, 256:512], op=Alu.add)
            v.tensor_tensor(out=det1[:, P, 1, :, :, :], in0=sw[:, :, ms, 0:256],
                            in1=sw[:, :, ms, 256:512], op=Alu.subtract)
            v.tensor_tensor(out=det1[:, P, 2, :, :, :], in0=dw[:, :, ms, 0:256],
                            in1=dw[:, :, ms, 256:512], op=Alu.subtract)

        def lvl1_ts(P):
            flat = det1[:, P, :, :, :, :].rearrange("p b ph m f -> p (b ph m f)")
            sc.activation(out=flat, in_=flat, func=Act.Abs)
            v.tensor_scalar(out=deadt[:, 0:3072], in0=flat, scalar1=THR[0],
                            scalar2=None, op0=Alu.min, op1=Alu.add,
                            accum_out=acc[:, 8 + P:9 + P])

        # ---------------- wavelet lvl2/lvl3 on PE ----------------
        def l1v(th, rp):
            return ll1[:, th, :, :].rearrange(
                "p m (k two) -> p m k two", two=2)[..., rp]

        def lvl2():
            pa0 = ppre.tile([128, 512], F32, name="pa0", tag="pre")
            pa1 = ppre.tile([128, 512], F32, name="pa1", tag="pre")
            pb = pc.tile([128, 2, 512], F32, name="pb", tag="ct")
            bands = ((pa0, None, 0, (0, 0)), (pa1, None, 1, (0, 0)),
                     (pb, 0, 0, (0, 1)), (pb, 1, 1, (0, 1)))
            # (tile, slot, sd, rp-neg pattern): band0=ll2(sum,+,+),
            # band1=det2[0](dif,+,+), band2=det2[1](sum,+,-), band3=det2[2](dif,+,-)
            for bi, (tile_, slot, sd, negs) in enumerate(bands):
                base = tile_[:, :] if slot is None else tile_[:, slot, :]
                dstv = base.rearrange("p (m k) -> p m k", m=IMG)
                k = 0
                for rp in range(2):
                    for th in range(2):
                        nc.tensor.matmul(
                            dstv, stats[:, st_l2(sd, th, negs[rp]):
                                        st_l2(sd, th, negs[rp]) + 128],
                            l1v(th, rp), start=(k == 0), stop=(k == 3))
                        k += 1
            sc.activation(out=ll2[:, :, :],
                          in_=pa0[:, :].rearrange("p (m k) -> p m k", m=IMG),
                          func=Act.Copy)
            sc.activation(out=det2[:, 0, :, :],
                          in_=pa1[:, :].rearrange("p (m k) -> p m k", m=IMG),
                          func=Act.Abs)
            sc.activation(out=det2[:, 1, :, :],
                          in_=pb[:, 0, :].rearrange("p (m k) -> p m k", m=IMG),
                          func=Act.Abs)
            sc.activation(out=det2[:, 2, :, :],
                          in_=pb[:, 1, :].rearrange("p (m k) -> p m k", m=IMG),
                          func=Act.Abs)
            flat = det2[:, :, :, :].rearrange("p b m f -> p (b m f)")
            v.tensor_scalar(out=deadt[:, 0:1536], in0=flat, scalar1=THR[1],
                            scalar2=None, op0=Alu.min, op1=Alu.add,
                            accum_out=acc[:, 10:11])

        def l2v(rp):
            return ll2[:, :, :].rearrange(
                "p m (k two) -> p m k two", two=2)[..., rp]

        def lvl3():
            p3a = ppre.tile([128, 512], F32, name="p3a", tag="pre")
            p3b = ppre.tile([128, 512], F32, name="p3b", tag="pre")
            regions = (p3a[:, 0:256], p3a[:, 256:512], p3b[:, 0:256])
            for b in range(3):
                sd = (1, 0, 1)[b]
                negs = ((0, 0), (0, 1), (0, 1))[b]
                dstv = regions[b].rearrange("p (m k) -> p m k", m=IMG)
                for rp in range(2):
                    nc.tensor.matmul(
                        dstv, stats[:, st_l3(sd, negs[rp]):
                                    st_l3(sd, negs[rp]) + 128],
                        l2v(rp), start=(rp == 0), stop=(rp == 1))
                sc.activation(out=det3[:, b, :, :], in_=dstv, func=Act.Abs)
            flat = det3[:, :, :, :].rearrange("p b m f -> p (b m f)")
            v.tensor_scalar(out=deadt[:, 0:768], in0=flat, scalar1=THR[2],
                            scalar2=None, op0=Alu.min, op1=Alu.add,
                            accum_out=acc[:, 11:12])

        # ---------------- main schedule ----------------
        for m in range(IMG):
            conv_unit(m, 0)
            conv_unit(m, 1)
            if m in (0, 2):
                min_img(m)
            if m == 1:
                shuffle_pair(0)
                lvl1_tt(0)
            if m == 2:
                # delta' = Gv/1.25 row factor: d + 0.3*(d shifted +-1)
                v.tensor_scalar(out=dtmp[:, :, :, :], in0=xst[:, 8:10, :, :],
                                scalar1=0.3, scalar2=None, op0=Alu.mult)
                v.tensor_tensor(out=dlp[:, :, :, 1:257],
                                in0=dtmp[:, :, :, 0:256],
                                in1=xst[:, 8:10, :, 1:257], op=Alu.add)
                v.tensor_tensor(out=dlp[:, :, :, 1:257],
                                in0=dtmp[:, :, :, 2:258],
                                in1=dlp[:, :, :, 1:257], op=Alu.add)
                lvl1_ts(0)
            if m == 3:
                shuffle_pair(1)
                lvl1_tt(1)
                lvl1_ts(1)
        for m in range(IMG):
            c_unit(m, 0)
            c_unit(m, 1)
            delta_img(m)
        lvl2()
        lvl3()

        nc.sync.dma_start(out=outh.ap(), in_=acc[:, :])

    import os
    if not os.environ.get("SKIP_WAIT_SPLIT"):
        _split_multiwaits(nc, mybir)
    return nc


def _split_multiwaits(nc, mybir):
    """HW instructions support exactly ONE sync-wait; split extras into
    standalone Drains."""
    for f in nc.m.functions:
        for bb in f.blocks:
            i = 0
            while i < len(bb.instructions):
                ins = bb.instructions[i]
                si = getattr(ins, "sync_info", None)
                if si is not None and si.on_wait and len(si.on_wait) > 1:
                    waits = list(si.on_wait)
                    for w in waits[:-1]:
                        d = mybir.InstDrain(
                            name=nc.get_next_instruction_name(),
                            ins=[], outs=[], bass_is_fusable=False)
                        d.engine = ins.engine
                        d.sync_info = mybir.SyncInfo(on_wait=[w], on_update=[])
                        bb.instructions.insert(i, d)
                        i += 1
                    ins.sync_info = mybir.SyncInfo(
                        on_wait=[waits[-1]], on_update=list(si.on_update))
                i += 1


def _get_nc():
    if "nc" not in _CACHE:
        _CACHE["nc"] = _build()
    return _CACHE["nc"]


def make_in_maps(noisy_input, weight):
    x = np.asarray(noisy_input, dtype=np.float32).reshape(B_TOTAL, H, W)
    stats = _build_stats(weight)
    maps = []
    for c in range(N_CORES):
        xs = np.zeros((128, NPLANE, IMG, 258), dtype=np.float16)
        for m in range(IMG):
            img = x[c * IMG + m]
            p0 = img[0::2, 0::2].astype(np.float64)
            p3 = img[1::2, 1::2].astype(np.float64)
            qc = 2.0 * p0 + p3
            dl = p0 - p3
            for base, ph, wins in ((0, p0, True), (4, qc, True),
                                   (8, dl, False)):
                pt = np.ascontiguousarray(ph.T)          # [col, row]
                st = np.concatenate([pt[:, :1], pt, pt[:, -1:]], axis=1)
                xs[:, base + 0, m, :] = st[0:128]
                xs[:, base + 1, m, :] = st[128:256]
                if wins:
                    xs[:, base + 2, m, :] = st[62:190]   # Wa
                    xs[:, base + 3, m, :] = st[126:254]  # Wb
        maps.append({"xs": xs, "st": stats})
    return maps


def _host_combine(parts):
    S_c = S_d = 0.0
    wav = np.zeros(3)
    for p in parts:
        q = p.astype(np.float64)
        S_c += q[:, 0:8].sum()
        wav[0] += q[:, 8:10].sum()
        wav[1] += q[:, 10].sum()
        wav[2] += q[:, 11].sum()
        S_d += q[:, 12:16].sum()
    N = B_TOTAL * H * W
    n2n = (3.0 * S_c + (2.0 / 3.0) * S_d) / N
    wtot = 0.0
    for j in (1, 2, 3):
        lvl = 3 - j + 1
        Nj = B_TOTAL * (H // 2 ** j) ** 2 * 3
        wtot += (1.0 / lvl) * (wav[j - 1] / (2.0 ** j)) / Nj
    return np.float32(n2n + WAVELET_WEIGHT * wtot)


def kernel(noisy_input, weight):
    from concourse.bass_utils import run_bass_kernel_spmd
    nc = _get_nc()
    in_maps = make_in_maps(noisy_input, weight)
    res = run_bass_kernel_spmd(nc, in_maps, list(range(N_CORES)))
    return _host_combine([r["res"] for r in res.results])


# revision 31
# speedup vs baseline: 1.5065x; 1.0366x over previous
"""Trainium2 Bass kernel for nn_CombinedN2NWaveletLoss — v2 redesign.

Loss algebra: rec + 2*reg = 3*mean((out-c)^2) + (2/3)*mean((U delta U^T)^2)
with c = U qc U^T / 3, qc = 2*p0 + p3, delta = p0 - p3.  The delta term is
computed on the coarse grid via the Gram quadratic form sum(delta' * (G d G))
with G = U^T U (tridiagonal: diag 1.25, off 0.375, clamped edges); the row
factor (Gv/1.25, exact at the clamped edges thanks to dup guards) is staged
from the host, the column factor is 2 main + 2 halo PE matmuls per image.

Layout: transposed; partitions carry image COLUMNS, free dim carries rows.
The vertical (row) 2x upsample is folded into the PE stationaries as 3
coarse row-taps (T[par][dn] = sum_dy rho[par,dy,dn] * (K_dy @ U)), so the
conv consumes raw p0 directly.  Spurious conv zero-pad terms at fine rows
0/511 are removed by an extra 1-column matmul inside each accumulation
group.  Eviction: ACT Relu (psum->fp16) + one DVE min(.,1) per image at 4x.
The c field uses 2 row-taps (1/3 folded in) plus -I matmuls on clipped out,
evicted with ACT Square+accum.  Wavelet lvl1: batched shuffle DMAs + DVE
butterflies + ACT Abs + DVE min/accum.  lvl2/lvl3: entirely on PE via
(P_E +- P_O) permutation-butterfly stationaries with row-pair accumulation,
ACT Abs-fused psum evictions, DVE min/accum.
"""

import numpy as np

B_TOTAL = 32
N_CORES = 8
IMG = 4
H = W = 512
HC = 256
THRESHOLD = 50.0 / 255.0
GAMMA = 2.0
WAVELET_WEIGHT = 0.05
WIN = (0, 62, 126, 128)          # stationary col-window start per chunk t
PAIRS = ((0, 3), (1, 2))         # chunk pairs per unit half
MOVP = (0, 2, 3, 1)              # p0 moving plane per chunk t
CPLANE = (4, 6, 7, 5)            # qc moving plane per chunk t
NPLANE = 12                      # p0 b0,b1,Wa,Wb; qc b0,b1,Wa,Wb; d b0,b1; d' b0,b1
NACC = 16                        # 0..7 c-units, 8..9 lvl1 pairs, 10 lvl2, 11 lvl3, 12..15 delta
CTAPS = (((-1, 0.25), (0, 0.75)), ((0, 0.75), (1, 0.25)))

# stationary pack indices; conv+corr blocks grouped by unit pair so the
# first DMA covers unit half 0 (chunks t0, t3)
_PPOS = (0, 2, 3, 1)      # t -> position in pack


def st_conv(t, par, dni):
    return (_PPOS[t] * 8 + par * 3 + dni) * 128

def st_corr(t, e):
    return (_PPOS[t] * 8 + 6 + e) * 128

def st_c(t, par, j):
    # Tc[0][0]=Tc[1][1]=(0.25/3)U-window, Tc[0][1]=Tc[1][0]=(0.75/3)U-window:
    # only 2 distinct blocks per chunk.
    big = 1 if (par == 0) == (j == 1) else 0
    return (32 + t * 2 + big) * 128

ST_I = 40 * 128

def st_g(ch, h):
    return (41 + ch * 2 + h) * 128

def st_l2(sd, th, neg):              # sd: 0=sum(P_E+P_O) 1=dif; neg: rp sign
    return (45 + sd * 2 + th + 4 * neg) * 128

def st_l3(sd, neg):
    return (53 + sd + 2 * neg) * 128

NSTAT = 57

_CACHE = {}


def _brev6(q):
    return int(f"{q:06b}"[::-1], 2)


def _lanes(t):
    idx = np.empty(128, dtype=np.int64)
    for q in range(128):
        jp = 64 * t + _brev6(q % 64)
        idx[q] = 2 * jp + (0 if q < 64 else 1)
    return idx


def _upsample_matrix():
    U = np.zeros((H, HC), dtype=np.float64)
    for j in range(H):
        src = (j + 0.5) / 2.0 - 0.5
        k0 = int(np.floor(src))
        frac = src - k0
        for k, wgt in ((k0, 1 - frac), (k0 + 1, frac)):
            U[j, min(max(k, 0), HC - 1)] += wgt
    return U


def _conv_colops(w):
    U = _upsample_matrix()
    wm = np.asarray(w, dtype=np.float64).reshape(3, 3)
    Nm = []
    for dy in (-1, 0, 1):
        K = np.zeros((H, H))
        for j in range(H):
            for dx in (-1, 0, 1):
                if 0 <= j + dx < H:
                    K[j, j + dx] = wm[dy + 1, dx + 1]
        Nm.append(K @ U)
    return Nm


def _perm_l2(eo):
    """P with P[src_lane, dst] = 1: E2/O2[32tv+s] <- ll1[(tv&1)*64+32*eo+s]."""
    P = np.zeros((128, 128))
    for tv in range(4):
        for s in range(32):
            P[(tv & 1) * 64 + 32 * eo + s, 32 * tv + s] = 1.0
    return P


def _perm_l2_th(eo, th):
    """th-masked: only tv with tv>>1 == th contribute."""
    P = np.zeros((128, 128))
    for tv in range(4):
        if (tv >> 1) != th:
            continue
        for s in range(32):
            P[(tv & 1) * 64 + 32 * eo + s, 32 * tv + s] = 1.0
    return P


def _perm_l3(eo):
    P = np.zeros((128, 128))
    for tv in range(4):
        for s in range(16):
            P[32 * tv + 16 * eo + s, 16 * tv + s] = 1.0
    return P


def _build_stats(w):
    Nm = _conv_colops(w)
    rho = np.zeros((2, 3, 3))
    rho[0, 0, 0] = 0.75; rho[0, 0, 1] = 0.25
    rho[0, 1, 0] = 0.25; rho[0, 1, 1] = 0.75
    rho[0, 2, 1] = 0.75; rho[0, 2, 2] = 0.25
    rho[1, 0, 0] = 0.25; rho[1, 0, 1] = 0.75
    rho[1, 1, 1] = 0.75; rho[1, 1, 2] = 0.25
    rho[1, 2, 1] = 0.25; rho[1, 2, 2] = 0.75
    T = [[sum(rho[par, dyi, dni] * Nm[dyi] for dyi in range(3))
          for dni in range(3)] for par in range(2)]
    U = _upsample_matrix()
    G125 = 1.25 * (U.T @ U)

    out = np.zeros((128, NSTAT * 128), dtype=np.float32)
    for t in range(4):
        J = _lanes(t)
        for par in range(2):
            for dni in range(3):
                k = st_conv(t, par, dni)
                out[:, k:k + 128] = T[par][dni][J, WIN[t]:WIN[t] + 128].T
        for big, r in ((0, 0.25), (1, 0.75)):
            k = (32 + t * 2 + big) * 128
            out[:, k:k + 128] = (r / 3.0) * U[J, WIN[t]:WIN[t] + 128].T
        # conv edge corrections (zero-pad spurious terms), 1-col matmuls
        for e, Ncor in ((0, Nm[0]), (1, Nm[2])):
            k = st_corr(t, e)
            out[:, k:k + 128] = -Ncor[J, WIN[t]:WIN[t] + 128].T
    out[:, ST_I:ST_I + 128] = -np.eye(128)
    for ch in range(2):
        a = 128 * ch
        b = 128 * (1 - ch)
        out[:, st_g(ch, 0):st_g(ch, 0) + 128] = G125[a:a + 128, a:a + 128]
        out[:, st_g(ch, 1):st_g(ch, 1) + 128] = G125[b:b + 128, a:a + 128]
    # lvl2/lvl3 butterfly stationaries
    for th in range(2):
        S = _perm_l2_th(0, th)
        O = _perm_l2_th(1, th)
        for neg in range(2):
            sgn = -1.0 if neg else 1.0
            out[:, st_l2(0, th, neg):st_l2(0, th, neg) + 128] = sgn * (S + O)
            out[:, st_l2(1, th, neg):st_l2(1, th, neg) + 128] = sgn * (S - O)
    S3 = _perm_l3(0)
    O3 = _perm_l3(1)
    for neg in range(2):
        sgn = -1.0 if neg else 1.0
        out[:, st_l3(0, neg):st_l3(0, neg) + 128] = sgn * (S3 + O3)
        out[:, st_l3(1, neg):st_l3(1, neg) + 128] = sgn * (S3 - O3)
    return out.astype(np.float16)


def _build():
    import concourse.bass as bass
    import concourse.mybir as mybir
    import concourse.tile as tile
    from contextlib import ExitStack

    dt = mybir.dt
    Alu = mybir.AluOpType
    Act = mybir.ActivationFunctionType
    F16, F32 = dt.float16, dt.float32

    T = THRESHOLD
    THR = (T / 4 * 2, T / 2 * 4, T * 8)

    nc = bass.Bass("TRN2", target_bir_lowering=False, debug=False,
                   num_devices=N_CORES)
    xsh = nc.dram_tensor("xs", [128, NPLANE, IMG, 258], F16, kind="ExternalInput")
    sth = nc.dram_tensor("st", [128, NSTAT * 128], F16, kind="ExternalInput")
    outh = nc.dram_tensor("res", [128, NACC], F32, kind="ExternalOutput")

    with tile.TileContext(nc) as tc, ExitStack() as ctx:
        v = nc.vector
        sc = nc.scalar

        pp = ctx.enter_context(tc.tile_pool(name="persist", bufs=1))
        xst = pp.tile([128, NPLANE, IMG, 258], F16, tag="xst")
        stats = pp.tile([128, NSTAT * 128], F16, tag="stats")
        out = pp.tile([128, 4, IMG, 512], F16, tag="out")     # [slot=t, m]
        Ee = pp.tile([128, 2, IMG, 512], F16, tag="Ee")       # [ph, m]
        Oo = pp.tile([128, 2, IMG, 512], F16, tag="Oo")
        sw = pp.tile([128, 2, IMG, 512], F16, tag="sw")       # [ph, m]
        dw = pp.tile([128, 2, IMG, 512], F16, tag="dw")
        ll1 = pp.tile([128, 2, IMG, 256], F16, tag="ll1")     # [th, m]
        det1 = pp.tile([128, 2, 3, 2, 2, 256], F16, tag="det1")  # [P, b, ph, mp]
        ll2 = pp.tile([128, IMG, 128], F16, tag="ll2")
        det2 = pp.tile([128, 3, IMG, 128], F16, tag="det2")
        det3 = pp.tile([128, 3, IMG, 64], F16, tag="det3")
        deadc = pp.tile([128, 2, 512], F16, tag="deadc")
        deadw = pp.tile([128, 2, 256], F16, tag="deadw")
        deadt = pp.tile([128, 3072], F16, tag="deadt")
        acc = pp.tile([128, NACC], F32, tag="acc")
        warm = pp.tile([128, 512], F16, tag="warm")

        ppre = ctx.enter_context(tc.tile_pool(name="ppre", bufs=4, space="PSUM"))
        pc = ctx.enter_context(tc.tile_pool(name="pc", bufs=2, space="PSUM"))

        # -------- input DMAs: one FIFO queue, consumer order --------
        nc.sync.dma_start(out=stats[:, 0:16 * 128], in_=sth.ap()[:, 0:16 * 128])
        nc.sync.dma_start(out=xst[:, 0:2, :, :], in_=xsh.ap()[:, 0:2])
        nc.sync.dma_start(out=xst[:, 2:4, :, :], in_=xsh.ap()[:, 2:4])
        nc.sync.dma_start(out=stats[:, 16 * 128:32 * 128],
                          in_=sth.ap()[:, 16 * 128:32 * 128])
        nc.sync.dma_start(out=xst[:, 4:8, :, :], in_=xsh.ap()[:, 4:8])
        nc.sync.dma_start(out=xst[:, 8:12, :, :], in_=xsh.ap()[:, 8:12])
        nc.sync.dma_start(out=stats[:, 32 * 128:], in_=sth.ap()[:, 32 * 128:])

        v.memset(warm[:, :], 0.0)
        v.memset(acc[:, :], 0.0)

        # PE warm-up (p-state ramp) while inputs land
        wps = ppre.tile([128, 512], F32, name="wps", tag="pre")
        for _ in range(8):
            nc.tensor.matmul(wps[:, :], warm[:, 0:128], warm[:, :],
                             start=True, stop=True)

        # ---------------- conv units ----------------
        def conv_unit(m, half):
            pres = []
            for ti, t in enumerate(PAIRS[half]):
                pre = ppre.tile([128, 512], F32, name="pre", tag="pre")
                pres.append(pre)
                mv = xst[:, MOVP[t], m, :]
                for par in range(2):
                    # edge-correction 1-col matmul folded into the group:
                    # par0 fixes fine row 0 (free slot 0), par1 row 511 (511)
                    nc.tensor.matmul(
                        pre[:, 256 * par:256 * par + 256],
                        stats[:, st_conv(t, par, 0):st_conv(t, par, 0) + 128],
                        mv[:, 0:256], start=True, stop=False)
                    nc.tensor.matmul(
                        pre[:, 256 * par:256 * par + 256],
                        stats[:, st_conv(t, par, 1):st_conv(t, par, 1) + 128],
                        mv[:, 1:257], start=False, stop=False)
                    if par == 0:
                        nc.tensor.matmul(
                            pre[:, 0:1],
                            stats[:, st_corr(t, 0):st_corr(t, 0) + 128],
                            mv[:, 1:2], start=False, stop=False)
                    else:
                        nc.tensor.matmul(
                            pre[:, 511:512],
                            stats[:, st_corr(t, 1):st_corr(t, 1) + 128],
                            mv[:, 256:257], start=False, stop=False)
                    nc.tensor.matmul(
                        pre[:, 256 * par:256 * par + 256],
                        stats[:, st_conv(t, par, 2):st_conv(t, par, 2) + 128],
                        mv[:, 2:258], start=False, stop=True)
            if m in (1, 3):
                for ti, t in enumerate(PAIRS[half]):
                    v.tensor_scalar(out=out[:, t, m, :], in0=pres[ti][:, :],
                                    scalar1=0.0, scalar2=1.0,
                                    op0=Alu.max, op1=Alu.min)
            else:
                for ti, t in enumerate(PAIRS[half]):
                    sc.activation(out=out[:, t, m, :], in_=pres[ti][:, :],
                                  func=Act.Relu)

        def min_img(m):
            v.tensor_scalar(out=out[:, :, m, :], in0=out[:, :, m, :],
                            scalar1=1.0, scalar2=None, op0=Alu.min)

        # ---------------- c units ----------------
        def c_unit(m, half):
            ct = pc.tile([128, 2, 512], F32, name="ct", tag="ct")
            for ti, t in enumerate(PAIRS[half]):
                mv = xst[:, CPLANE[t], m, :]
                for par in range(2):
                    cols = slice(256 * par, 256 * par + 256)
                    for j, (dn, r) in enumerate(CTAPS[par]):
                        nc.tensor.matmul(ct[:, ti, cols],
                                         stats[:, st_c(t, par, j):st_c(t, par, j) + 128],
                                         mv[:, dn + 1:dn + 257],
                                         start=(j == 0), stop=False)
                    nc.tensor.matmul(ct[:, ti, cols],
                                     stats[:, ST_I:ST_I + 128],
                                     out[:, t, m, cols],
                                     start=False, stop=True)
            sc.activation(out=deadc[:, :, :], in_=ct[:, :, :], func=Act.Square,
                          accum_out=acc[:, 2 * m + half:2 * m + half + 1])

        # ---------------- delta term ----------------
        def delta_img(m):
            Wp = pc.tile([128, 2, 512], F32, name="wp", tag="ct")
            for ch in range(2):
                nc.tensor.matmul(Wp[:, ch, 0:256],
                                 stats[:, st_g(ch, 0):st_g(ch, 0) + 128],
                                 xst[:, 10 + ch, m, 1:257], start=True, stop=False)
                nc.tensor.matmul(Wp[:, ch, 0:256],
                                 stats[:, st_g(ch, 1):st_g(ch, 1) + 128],
                                 xst[:, 11 - ch, m, 1:257], start=False, stop=True)
            v.tensor_tensor(out=deadw[:, :, :], in0=xst[:, 8:10, m, 1:257],
                            in1=Wp[:, :, 0:256], op=Alu.mult)
            v.tensor_scalar(out=deadw[:, :, :], in0=deadw[:, :, :], scalar1=0.0,
                            scalar2=None, op0=Alu.bypass, op1=Alu.add,
                            accum_out=acc[:, 12 + m:13 + m])

        # ---------------- wavelet lvl1 ----------------
        outr = out[:, :, :, :].rearrange("p (sa sb) m f -> p sb sa (m f)",
                                         sa=2, sb=2)

        def shuffle_pair(P):
            fs = slice(1024 * P, 1024 * P + 1024)
            ms = slice(2 * P, 2 * P + 2)
            nc.sync.dma_start(out=Ee[0:64, :, ms, :], in_=outr[0:64, 0, :, fs])
            nc.sync.dma_start(out=Ee[64:128, :, ms, :], in_=outr[0:64, 1, :, fs])
            nc.sync.dma_start(out=Oo[0:64, :, ms, :], in_=outr[64:128, 0, :, fs])
            nc.sync.dma_start(out=Oo[64:128, :, ms, :], in_=outr[64:128, 1, :, fs])

        def lvl1_tt(P):
            ms = slice(2 * P, 2 * P + 2)
            eer = Ee[:, :, ms, :]
            oor = Oo[:, :, ms, :]
            v.tensor_tensor(out=sw[:, :, ms, :], in0=eer, in1=oor, op=Alu.add)
            v.tensor_tensor(out=dw[:, :, ms, :], in0=eer, in1=oor, op=Alu.subtract)
            v.tensor_tensor(out=ll1[:, :, ms, :], in0=sw[:, :, ms, 0:256],
                            in1=sw[:, :, ms, 256:512], op=Alu.add)
            v.tensor_tensor(out=det1[:, P, 0, :, :, :], in0=dw[:, :, ms, 0:256],
                            in1=dw[:, :, ms, 256:512], op=Alu.add)
            v.tensor_tensor(out=det1[:, P, 1, :, :, :], in0=sw[:, :, ms, 0:256],
                            in1=sw[:, :, ms, 256:512], op=Alu.subtract)
            v.tensor_tensor(out=det1[:, P, 2, :, :, :], in0=dw[:, :, ms, 0:256],
                            in1=dw[:, :, ms, 256:512], op=Alu.subtract)

        def lvl1_ts(P):
            flat = det1[:, P, :, :, :, :].rearrange("p b ph m f -> p (b ph m f)")
            sc.activation(out=flat, in_=flat, func=Act.Abs)
            v.tensor_scalar(out=deadt[:, 0:3072], in0=flat, scalar1=THR[0],
                            scalar2=None, op0=Alu.min, op1=Alu.add,
                            accum_out=acc[:, 8 + P:9 + P])

        # ---------------- wavelet lvl2/lvl3 on PE ----------------
        def l1v(th, rp):
            return ll1[:, th, :, :].rearrange(
                "p m (k two) -> p m k two", two=2)[..., rp]

        def lvl2():
            pa0 = ppre.tile([128, 512], F32, name="pa0", tag="pre")
            pa1 = ppre.tile([128, 512], F32, name="pa1", tag="pre")
            pb = pc.tile([128, 2, 512], F32, name="pb", tag="ct")
            bands = ((pa0, None, 0, (0, 0)), (pa1, None, 1, (0, 0)),
                     (pb, 0, 0, (0, 1)), (pb, 1, 1, (0, 1)))
            # (tile, slot, sd, rp-neg pattern): band0=ll2(sum,+,+),
            # band1=det2[0](dif,+,+), band2=det2[1](sum,+,-), band3=det2[2](dif,+,-)
            for bi, (tile_, slot, sd, negs) in enumerate(bands):
                base = tile_[:, :] if slot is None else tile_[:, slot, :]
                dstv = base.rearrange("p (m k) -> p m k", m=IMG)
                k = 0
                for rp in range(2):
                    for th in range(2):
                        nc.tensor.matmul(
                            dstv, stats[:, st_l2(sd, th, negs[rp]):
                                        st_l2(sd, th, negs[rp]) + 128],
                            l1v(th, rp), start=(k == 0), stop=(k == 3))
                        k += 1
            sc.activation(out=ll2[:, :, :],
                          in_=pa0[:, :].rearrange("p (m k) -> p m k", m=IMG),
                          func=Act.Copy)
            sc.activation(out=det2[:, 0, :, :],
                          in_=pa1[:, :].rearrange("p (m k) -> p m k", m=IMG),
                          func=Act.Abs)
            sc.activation(out=det2[:, 1, :, :],
                          in_=pb[:, 0, :].rearrange("p (m k) -> p m k", m=IMG),
                          func=Act.Abs)
            sc.activation(out=det2[:, 2, :, :],
                          in_=pb[:, 1, :].rearrange("p (m k) -> p m k", m=IMG),
                          func=Act.Abs)
            flat = det2[:, :, :, :].rearrange("p b m f -> p (b m f)")
            v.tensor_scalar(out=deadt[:, 0:1536], in0=flat, scalar1=THR[1],
                            scalar2=None, op0=Alu.min, op1=Alu.add,
                            accum_out=acc[:, 10:11])

        def l2v(rp):
            return ll2[:, :, :].rearrange(
                "p m (k two) -> p m k two", two=2)[..., rp]

        def lvl3():
            p3a = ppre.tile([128, 512], F32, name="p3a", tag="pre")
            p3b = ppre.tile([128, 512], F32, name="p3b", tag="pre")
            regions = (p3a[:, 0:256], p3a[:, 256:512], p3b[:, 0:256])
            for b in range(3):
                sd = (1, 0, 1)[b]
                negs = ((0, 0), (0, 1), (0, 1))[b]
                dstv = regions[b].rearrange("p (m k) -> p m k", m=IMG)
                for rp in range(2):
                    nc.tensor.matmul(
                        dstv, stats[:, st_l3(sd, negs[rp]):
                                    st_l3(sd, negs[rp]) + 128],
                        l2v(rp), start=(rp == 0), stop=(rp == 1))
                sc.activation(out=det3[:, b, :, :], in_=dstv, func=Act.Abs)
            flat = det3[:, :, :, :].rearrange("p b m f -> p (b m f)")
            v.tensor_scalar(out=deadt[:, 0:768], in0=flat, scalar1=THR[2],
                            scalar2=None, op0=Alu.min, op1=Alu.add,
                            accum_out=acc[:, 11:12])

        # ---------------- main schedule ----------------
        for P in range(2):
            m0, m1 = 2 * P, 2 * P + 1
            conv_unit(m0, 0)
            conv_unit(m1, 0)
            conv_unit(m0, 1)
            conv_unit(m1, 1)
            min_img(m0)
            shuffle_pair(P)
            lvl1_tt(P)
        for m in range(IMG):
            c_unit(m, 0)
            c_unit(m, 1)
            delta_img(m)
        lvl1_ts(0)
        lvl1_ts(1)
        lvl2()
        lvl3()

        nc.sync.dma_start(out=outh.ap(), in_=acc[:, :])

    import os
    if not os.environ.get("SKIP_WAIT_SPLIT"):
        _split_multiwaits(nc, mybir)
    return nc


def _split_multiwaits(nc, mybir):
    """HW instructions support exactly ONE sync-wait; split extras into
    standalone Drains."""
    for f in nc.m.functions:
        for bb in f.blocks:
            i = 0
            while i < len(bb.instructions):
                ins = bb.instructions[i]
                si = getattr(ins, "sync_info", None)
                if si is not None and si.on_wait and len(si.on_wait) > 1:
                    waits = list(si.on_wait)
                    for w in waits[:-1]:
                        d = mybir.InstDrain(
                            name=nc.get_next_instruction_name(),
                            ins=[], outs=[], bass_is_fusable=False)
                        d.engine = ins.engine
                        d.sync_info = mybir.SyncInfo(on_wait=[w], on_update=[])
                        bb.instructions.insert(i, d)
                        i += 1
                    ins.sync_info = mybir.SyncInfo(
                        on_wait=[waits[-1]], on_update=list(si.on_update))
                i += 1


def _get_nc():
    if "nc" not in _CACHE:
        _CACHE["nc"] = _build()
    return _CACHE["nc"]


def make_in_maps(noisy_input, weight):
    x = np.asarray(noisy_input, dtype=np.float32).reshape(B_TOTAL, H, W)
    stats = _build_stats(weight)
    maps = []
    for c in range(N_CORES):
        xs = np.zeros((128, NPLANE, IMG, 258), dtype=np.float16)
        for m in range(IMG):
            img = x[c * IMG + m]
            p0 = img[0::2, 0::2].astype(np.float64)
            p3 = img[1::2, 1::2].astype(np.float64)
            qc = 2.0 * p0 + p3
            dl = p0 - p3
            g = np.concatenate([dl[:1], dl, dl[-1:]], axis=0)
            dp = g[1:257] + 0.3 * (g[0:256] + g[2:258])   # Gv/1.25 row factor
            for base, ph, wins in ((0, p0, True), (4, qc, True),
                                   (8, dl, False)):
                pt = np.ascontiguousarray(ph.T)          # [col, row]
                st = np.concatenate([pt[:, :1], pt, pt[:, -1:]], axis=1)
                xs[:, base + 0, m, :] = st[0:128]
                xs[:, base + 1, m, :] = st[128:256]
                if wins:
                    xs[:, base + 2, m, :] = st[62:190]   # Wa
                    xs[:, base + 3, m, :] = st[126:254]  # Wb
            dpt = np.ascontiguousarray(dp.T)
            xs[:, 10, m, 1:257] = dpt[0:128]
            xs[:, 11, m, 1:257] = dpt[128:256]
        maps.append({"xs": xs, "st": stats})
    return maps


def _host_combine(parts):
    S_c = S_d = 0.0
    wav = np.zeros(3)
    for p in parts:
        q = p.astype(np.float64)
        S_c += q[:, 0:8].sum()
        wav[0] += q[:, 8:10].sum()
        wav[1] += q[:, 10].sum()
        wav[2] += q[:, 11].sum()
        S_d += q[:, 12:16].sum()
    N = B_TOTAL * H * W
    n2n = (3.0 * S_c + (2.0 / 3.0) * S_d) / N
    wtot = 0.0
    for j in (1, 2, 3):
        lvl = 3 - j + 1
        Nj = B_TOTAL * (H // 2 ** j) ** 2 * 3
        wtot += (1.0 / lvl) * (wav[j - 1] / (2.0 ** j)) / Nj
    return np.float32(n2n + WAVELET_WEIGHT * wtot)


def kernel(noisy_input, weight):
    from concourse.bass_utils import run_bass_kernel_spmd
    nc = _get_nc()
    in_maps = make_in_maps(noisy_input, weight)
    res = run_bass_kernel_spmd(nc, in_maps, list(range(N_CORES)))
    return _host_combine([r["res"] for r in res.results])
